# revision 108
# baseline (speedup 1.0000x reference)
"""Autoformer encoder (nn_Autoformer_11441792876586) on 8 TRN2 NeuronCores.

Strategy: data-parallel over batch (4 batches/core). Per core, everything runs
in feature-major layout (channels on partitions, time on free dim):
  - all GEMMs: W stationary (din,dout)-tiles, activations moving -> fp16 in,
    fp32 PSUM accumulate
  - AutoCorrelation mean_corr via Gram matrix M' = K^T Q (c-contraction),
    block-diagonal sums C'_j accumulated straight from PSUM, DRAM shear read
    with ASCENDING element stride, ones-vector matmul partition reduction;
    top-8 via DVE Max8/MaxIndex; softmax on ACT/DVE
  - the weighted time-rolls sum_k tc_k*roll(V@Wo, d_k) run on the PE as a
    block-circulant matmul: V@Wo is produced TIME-major (voT, stationary =
    v16 time-slices), the 8 distinct 128x128 circulant blocks B_g[u,v] =
    sum_k tc_k*[(128g+u-v) mod 1024 == d_k] are built with one DVE
    is_equal+mult tensor_scalar per k against a host iota constant UG, and
    out[c, 128T+v] = sum_g voT[(T+g)%8]^T B_g accumulates in PSUM. This
    keeps gpsimd (Q7 ~27us/call dispatch stalls) entirely out of the kernel.
    bo is dropped: decomp(h + a + const) == decomp(h + a) since the
    edge-replicated moving average maps time-constants to themselves and
    softmax(tc) sums to 1.
  - all partition broadcasts via rank-1 ones-row matmuls (no gpsimd)
  - series_decomp via DVE prefix-scan cumsum + edge-replication corrections
  - residual stream h kept in f32 (the top-k selection has near-ties at
    ~1e-4 relative; an f16 stream drifts enough to flip them), cast to f16
    tiles right before each GEMM phase on the scalar engine
  - final my_Layernorm factored so the projection GEMMs depend only on the
    h cast, never a z-normalized tensor: out = P - mean_t(P) + pb with
    P = inv*(h^T pw) - (inv*mu)*colsum(pw); the inv/mu rows reach
    time-major per-partition form via a small DRAM round trip
Scheduling: all (layer, batch) units are software-pipelined 2 deep with
engines' in-order streams in mind. Per unit the emission order is
[combine(u)] [final-proj tail] [chain1(u+1): shear-read + s_j + top-8 +
softmax] [stage_a(u+2): QKV/Gram/voT + weight DMAs on the gpsimd queue]
[chain2(u+1): broadcast + B-build] [back(u): decomp/FFN/decomp], so the
PE always has the next unit's GEMMs queued while a unit's correlation
chain (DMA round trip + DVE small ops) resolves, and the B-build lands
in the DVE queue a full unit before its combine. gpsimd is used ONLY to
issue weight-load DMA descriptors: every Q7 compute op measured 3-27us
of dispatch latency and repeatedly regressed the span.
"""
import sys
if "/opt/trn_rl_repo" not in sys.path:
    sys.path.insert(0, "/opt/trn_rl_repo")
import hashlib as _hashlib
import os as _os
import numpy as np

# The neuronxcc NEFF cache keys on the HLO module hash, which does NOT cover
# the embedded bass program — a stale cache silently serves NEFFs built from
# an older kernel revision. Pin the cache to a dir derived from this file's
# contents so any source change forces a recompile.
_os.environ["NEURON_COMPILE_CACHE_URL"] = (
    "/tmp/neuron_cache_"
    + _hashlib.md5(open(__file__, "rb").read()).hexdigest()[:16])

L, CIN, D, DFF = 1024, 64, 512, 2048
NL_TOT, KMA, TOPK, EPS = 3, 25, 6, 1e-5
NCORES = 8
PAD = (KMA - 1) // 2  # 12
NTC = 2               # 512-wide time chunks per 1024
_BUILD_CACHE = {}


def _build(nbatch, nlayers):
    import concourse.bass as bass
    from concourse.bass import _add_dep_helper
    import concourse.bacc as bacc
    import concourse.tile as tile
    import concourse.mybir as mybir
    from contextlib import ExitStack

    DT = mybir.dt
    AF = mybir.ActivationFunctionType
    ALU = mybir.AluOpType
    AX = mybir.AxisListType
    F16, F32 = DT.float16, DT.float32

    nc = bacc.Bacc("TRN2", target_bir_lowering=False, debug=False,
                   num_devices=NCORES)

    # ---------------- I/O ----------------
    x_e = nc.dram_tensor("x", [nbatch, 64, L + 2], F16, kind="ExternalInput")
    ug_e = nc.dram_tensor("ug", [128, 1024], F16, kind="ExternalInput")
    onesr_e = nc.dram_tensor("onesr", [1, 128], F32, kind="ExternalInput")
    wqkvo_e = nc.dram_tensor("wqkvo", [nlayers, 4, 4, 128, 512], F16, kind="ExternalInput")
    w1_e = nc.dram_tensor("w1", [nlayers, 4, 128, DFF], F16, kind="ExternalInput")
    w2_e = nc.dram_tensor("w2", [nlayers, 16, 128, 512], F16, kind="ExternalInput")
    ew_e = nc.dram_tensor("ew", [3, 64, 512], F16, kind="ExternalInput")
    bias_e = nc.dram_tensor("biases", [128, nlayers * 16], F32, kind="ExternalInput")
    projw_e = nc.dram_tensor("projw", [4, 128, 64], F16, kind="ExternalInput")
    pbt_e = nc.dram_tensor("pbt", [128, 64], F32, kind="ExternalInput")
    spwn_e = nc.dram_tensor("spwn", [128, 64], F32, kind="ExternalInput")
    onesk_e = nc.dram_tensor("onesk", [128, 1], F32, kind="ExternalInput")
    # final-LN row transpose scratch (parity per batch)
    dfin = nc.dram_tensor("dfin", [2, 2, 1024], F32)
    rampf_e = nc.dram_tensor("rampf", [128, PAD + 1], F32, kind="ExternalInput")
    rampb_e = nc.dram_tensor("rampb", [128, PAD], F32, kind="ExternalInput")
    ones_e = nc.dram_tensor("ones512", [128, 1], F16, kind="ExternalInput")
    out_e = nc.dram_tensor("out", [nbatch, L, 64], F16, kind="ExternalOutput")
    # internal DRAM shear buffers (alternating parity per batch)
    dsh = [nc.dram_tensor(f"dsh{p}", [8, 128, 512], F16) for p in range(2)]

    with tile.TileContext(nc) as tc, ExitStack() as ctx:
        pool = ctx.enter_context(tc.tile_pool(name="sb", bufs=1))
        wpool = ctx.enter_context(tc.tile_pool(name="wp", bufs=1))
        pq = ctx.enter_context(tc.tile_pool(name="pq", bufs=5, space="PSUM"))
        pf = ctx.enter_context(tc.tile_pool(name="pf", bufs=3, space="PSUM"))

        # ------------- persistent constants -------------
        # ug + ew first: ug feeds the PE warm-up immediately and ew feeds
        # the embed; the rest can land while those run
        ug_sb = pool.tile([128, 1024], F16, tag="ug")
        nc.sync.dma_start(ug_sb[:], ug_e.ap())
        # PE clock warm-up: ~5us of dummy matmuls overlapping the remaining
        # constant/input DMAs, so the embed and first QKV stream at 2.4GHz
        # instead of the 1.2GHz cold clock (PE ramps after ~4us sustained)
        warm = pool.tile([128, 512], F16, tag="cw16", name="warm")
        for wi in range(12):
            psw = pq.tile([128, 512], F32, tag="ps")
            nc.tensor.matmul(psw[:], ug_sb[:, 0:128], ug_sb[:, 0:512],
                             start=True, stop=True)
            if wi == 11:
                nc.scalar.activation(warm[:], psw[:], AF.Identity)
        # ew lives in the g16 slot: it is only read during the embed stage,
        # before the first FFN allocates g16
        ew_sb = pool.tile([64, 3 * 512], F16, tag="g16", name="ew_sb")
        nc.sync.dma_start(ew_sb[:].rearrange("p (j c) -> p j c", j=3),
                          ew_e.ap().rearrange("j p c -> p j c"))
        bias_sb = pool.tile([128, nlayers * 16], F32, tag="bias")
        nc.sync.dma_start(bias_sb[:], bias_e.ap())
        projw_sb = pool.tile([128, 4 * 64], F16, tag="projw")
        nc.sync.dma_start(projw_sb[:].rearrange("p (k c) -> p k c", k=4),
                          projw_e.ap().rearrange("k p c -> p k c"))
        pbt_sb = pool.tile([128, 64], F32, tag="pbt")
        nc.sync.dma_start(pbt_sb[:], pbt_e.ap())
        spwn_sb = pool.tile([128, 64], F32, tag="spwn")
        nc.sync.dma_start(spwn_sb[:], spwn_e.ap())
        onesk_sb = pool.tile([128, 1], F32, tag="onesk")
        nc.sync.dma_start(onesk_sb[:], onesk_e.ap())
        rampf_sb = pool.tile([128, PAD + 1], F32, tag="rampf")
        nc.sync.dma_start(rampf_sb[:], rampf_e.ap())
        rampb_sb = pool.tile([128, PAD], F32, tag="rampb")
        nc.sync.dma_start(rampb_sb[:], rampb_e.ap())
        ones_sb = pool.tile([128, 1], F16, tag="ones")
        nc.sync.dma_start(ones_sb[:], ones_e.ap())
        onesr_sb = pool.tile([1, 128], F32, tag="onesr")
        nc.sync.dma_start(onesr_sb[:], onesr_e.ap())

        zero_writes = {0: [], 1: []}
        shear_writes = [{}, {}]
        shear_reads = [{}, {}]
        fin_reads = {}

        # persistent per-batch h (f32, feature-major: c-tile m at cols [1024m))
        h_b = [pool.tile([128, 4 * L], F32, tag=f"h{b}", name=f"h{b}")
               for b in range(nbatch)]

        def cast_h16(b, tag):
            """f16 snapshot of h for GEMM consumption, on the scalar engine
            (ACT Identity rounds f32->f16 the same as a DVE copy, and the
            scalar engine has headroom while DVE is the bottleneck)."""
            h16 = pool.tile([128, 4 * L], F16, tag=tag, name="h16")
            for m in range(4):
                nc.scalar.activation(h16[:, m * L:(m + 1) * L],
                                     h_b[b][:, m * L:(m + 1) * L], AF.Identity)
            return h16

        def gemm_512(dst_sb, dst_col, w_sb, w_base, rhs_sb, psum_pool,
                     bias_ap=None, act=None, nk=4):
            """dst[:, dst_col + m*L + tc*512] = act(sum_k W[k,m]^T @ rhs[k,tc]) + bias
            W blocks at w_sb[:, w_base + 512k + 128m]; rhs c-tile k at rhs_sb cols
            [L*k], time chunk tc at [512tc]. dst layout: c-tile m at [L*m]."""
            for m in range(4):
                for t in range(NTC):
                    ps = psum_pool.tile([128, 512], F32, tag="ps")
                    for k in range(nk):
                        nc.tensor.matmul(
                            ps[:],
                            w_sb[:, w_base + 512 * k + 128 * m:
                                 w_base + 512 * k + 128 * m + 128],
                            rhs_sb[:, L * k + 512 * t: L * k + 512 * t + 512],
                            start=(k == 0), stop=(k == nk - 1))
                    col = dst_col + L * m + 512 * t
                    nc.scalar.activation(dst_sb[:, col:col + 512], ps[:],
                                         act or AF.Identity,
                                         bias=bias_ap[m] if bias_ap else 0.0)

        def bias_aps(l, w):
            return [bias_sb[:, l * 16 + w * 4 + m: l * 16 + w * 4 + m + 1]
                    for m in range(4)]

        def decomp(b):
            """h <- h - moving_average(h) with edge replication; h=(128,4L) f32.

            The cs tile holds the EXACT padded cumsum (cs[i] = sum of the
            first i entries of the edge-replicated sequence, up to a global
            constant that cancels in the windowed diff): front pad =
            (i-12)*x0 via one tensor_scalar on an iota constant, scan with
            zero initial, back pad = total + k*x_last via one STT. No
            separate edge corrections or saved edge columns needed.
            DVE, not gpsimd: decomp is on the critical path and the Q7
            handoff adds ~2-4us latency per call."""
            h = h_b[b]
            for m in range(4):
                hx = h[:, m * L:(m + 1) * L]
                cs = pool.tile([128, L + 2 * PAD + 4], F32, tag="cs",
                               name="cs")
                nc.vector.tensor_scalar(cs[:, 0:PAD + 1], rampf_sb[:],
                                        hx[:, 0:1], None, ALU.mult)
                nc.vector.tensor_tensor_scan(cs[:, PAD + 1:PAD + 1 + L], hx, hx,
                                             0.0, ALU.add, ALU.bypass)
                nc.vector.scalar_tensor_tensor(
                    cs[:, PAD + 1 + L:PAD + 1 + L + PAD], rampb_sb[:],
                    hx[:, L - 1:L],
                    cs[:, PAD + L:PAD + 1 + L].to_broadcast((128, PAD)),
                    ALU.mult, ALU.add)
                # windowed sum A[t] = cs[t+25] - cs[t]; h = hx - A/25
                tmp = pool.tile([128, L], F32, tag="tmp", name="tmp")
                nc.vector.tensor_tensor(tmp[:], cs[:, KMA:KMA + L], cs[:, 0:L],
                                        ALU.subtract)
                nc.vector.scalar_tensor_tensor(hx, tmp[:], -1.0 / KMA, hx,
                                               ALU.mult, ALU.add)

        # ================= embed =================
        for b in range(nbatch):
            x_sb = pool.tile([64, L + 2], F16, tag="cs", name="x_sb")
            nc.sync.dma_start(x_sb[:], x_e.ap()[b])
            for m in range(4):
                for t in range(NTC):
                    ps = pq.tile([128, 512], F32, tag="ps")
                    for j in range(3):
                        nc.tensor.matmul(
                            ps[:],
                            ew_sb[0:64, 512 * j + 128 * m: 512 * j + 128 * m + 128],
                            x_sb[0:64, j + 512 * t: j + 512 * t + 512],
                            start=(j == 0), stop=(j == 2))
                    nc.scalar.activation(
                        h_b[b][:, L * m + 512 * t: L * m + 512 * t + 512],
                        ps[:], AF.Identity)

        # zero the shear scratch in DRAM once; emitted AFTER the embed so
        # these 2MB of writes queue behind the x loads the head needs first
        # (they only have to precede the first shear read, ~100us in)
        zero_sb = pool.tile([128, 512], F16, tag="cs", name="zero_sb")
        nc.vector.memset(zero_sb[:], 0.0)
        for p in range(2):
            for j in range(8):
                zero_writes[p].append(
                    nc.sync.dma_start(dsh[p].ap()[j], zero_sb[:]))

        # ================= layers (flat (l, b) unit pipeline) =================
        def load_qkvo(l):
            """Weight loads ride the GpSimd DMA queue (not Sync), so a load
            whose WAR-wait on the previous layer's last reads hasn't cleared
            can't head-of-line-block the Sync queue's shear DMAs. The qkvo
            load is emitted two units ahead (WAR on voT(l-1, last) clears
            early); the ffn load is emitted at unit (l, 0) itself since its
            WAR only clears at FFN2(l-1, last)."""
            qkvo_sb = wpool.tile([128, 4 * 2048], F16, tag="qkvo")
            nc.gpsimd.dma_start(
                qkvo_sb[:].rearrange("p (w k c) -> p w k c", w=4, k=4),
                wqkvo_e.ap()[l].rearrange("w k p c -> p w k c"))
            return qkvo_sb

        def load_ffn(l):
            ffn_sb = wpool.tile([128, 16384], F16, tag="ffn")
            nc.gpsimd.dma_start(
                ffn_sb[:, 0:8192].rearrange("p (k c) -> p k c", k=4),
                w1_e.ap()[l].rearrange("k p c -> p k c"))
            nc.gpsimd.dma_start(
                ffn_sb[:, 8192:16384].rearrange("p (k c) -> p k c", k=16),
                w2_e.ap()[l].rearrange("k p c -> p k c"))
            return ffn_sb

        if True:
            def stage_a(l, b, qkvo_sb):
                """PE-heavy front half: QKV gemms, Gram + shear write, voT.
                Emitted one batch AHEAD of stage_b(b-1) so the PE stream has
                queued work while b-1's correlation chain (DMA/DVE/ACT small
                ops) resolves."""
                h16 = cast_h16(b, "h16q")
                q16 = pool.tile([128, 4 * L], F16, tag="q16")
                k16 = pool.tile([128, 4 * L], F16, tag="k16")
                v16 = pool.tile([128, 4 * L], F16, tag="v16", name="v16")
                gemm_512(q16, 0, qkvo_sb, 0, h16, pq, bias_aps(l, 0))
                gemm_512(k16, 0, qkvo_sb, 2048, h16, pq, bias_aps(l, 1))
                gemm_512(v16, 0, qkvo_sb, 4096, h16, pq, bias_aps(l, 2))

                # ---- Gram M'_i = K_i^T Q (fp32 psum chunks, rounded to f16
                # per chunk); C'_j then summed from the f16 chunks on DVE:
                # C'_j[p,u] = sum_i M'_i[p, 128*((i+j)%8)+u]. A j-major
                # all-PSUM variant (32 128-wide matmuls per j) measured
                # +0.36ms: 128-wide moving exposes the 87ns LDWEIGHTS that a
                # 512-wide stream hides, so keep the 512-wide chunks.
                cacc = pool.tile([128, 1024], F32, tag="cacc")
                cw16 = pool.tile([128, 1024], F16, tag="cw16", name="cw16")
                for i in range(8):
                    for t2 in range(2):
                        psm = pq.tile([128, 512], F32, tag="ps")
                        for kc in range(4):
                            nc.tensor.matmul(
                                psm[:],
                                k16[:, L * kc + 128 * i: L * kc + 128 * i + 128],
                                q16[:, L * kc + 512 * t2: L * kc + 512 * t2 + 512],
                                start=(kc == 0), stop=(kc == 3))
                        chunk16 = pool.tile([128, 512], F16, tag="chunk16",
                                            bufs=2, name="chunk16")
                        nc.scalar.activation(chunk16[:], psm[:], AF.Identity)
                        for qb in range(4):
                            jq = 4 * t2 + qb          # q-time block in this chunk
                            j = (jq - i) % 8          # C'_j slice it feeds
                            blk = chunk16[:, 128 * qb:128 * qb + 128]
                            dst32 = cacc[:, 128 * j:128 * j + 128]
                            if i == 0:
                                nc.vector.tensor_copy(dst32, blk)
                            elif i < 7:
                                nc.vector.tensor_tensor(dst32, dst32, blk,
                                                        ALU.add)
                            else:
                                nc.vector.tensor_tensor(
                                    cw16[:, 128 * j:128 * j + 128], dst32, blk,
                                    ALU.add)
                # all 8 C'_j blocks in ONE 3-dim DMA (dst order (p, j, c))
                shw = bass.AP(tensor=dsh[b % 2].ap().tensor, offset=256,
                              ap=[[512, 128], [128 * 512, 8], [1, 128]])
                wr = nc.sync.dma_start(
                    shw, cw16[:].rearrange("p (j c) -> p j c", j=8))
                for j in range(8):
                    for prd in shear_reads[b % 2].get(j, []):
                        _add_dep_helper(wr.ins, prd.ins, sync=True,
                                        reason="shear WAR")
                    shear_writes[b % 2][j] = wr

                # voT = (v @ wo) TIME-major: voT_T[u, c] = sum_cin
                # v16[cin, 128T+u] * wo[cin, c]; stationary = v16 time-slices,
                # moving = wo [cin, 512 cout] (contiguous in qkvo_sb). bo is
                # dropped (decomp kills time-constant shifts, softmax sums
                # to 1). Emitted after the Gram so corr progress is preferred.
                vot = pool.tile([128, 8 * 512], F16, tag="vo2", bufs=2,
                                name="vot")
                for tb in range(8):
                    psv = pq.tile([128, 512], F32, tag="ps")
                    for kc in range(4):
                        nc.tensor.matmul(
                            psv[:],
                            v16[:, L * kc + 128 * tb: L * kc + 128 * tb + 128],
                            qkvo_sb[:, 6144 + 512 * kc: 6144 + 512 * kc + 512],
                            start=(kc == 0), stop=(kc == 3))
                    nc.scalar.activation(vot[:, 512 * tb:512 * tb + 512],
                                         psv[:], AF.Identity)
                return vot

            def stage_b_chain(b):
                # ---- shear read T[p, (j,u')] = D[j, p, p + 128 + u'] ----
                # ascending inner stride: 256 contiguous f16 = 512B/partition
                t16 = pool.tile([128, 8 * 256], F16, tag="t16", name="t16")
                shear_in = bass.AP(
                    tensor=dsh[b % 2].ap().tensor,
                    offset=128,
                    ap=[[513, 128], [128 * 512, 8], [1, 256]])
                rd = nc.sync.dma_start(
                    t16[:].rearrange("p (j t) -> p j t", j=8), shear_in)
                for j in range(8):
                    _add_dep_helper(rd.ins, shear_writes[b % 2][j].ins,
                                    sync=True, reason="shear RAW")
                    shear_reads[b % 2].setdefault(j, []).append(rd)
                for zw in zero_writes[b % 2]:
                    _add_dep_helper(rd.ins, zw.ins, sync=True,
                                    reason="shear zero RAW")

                # ---- s_j = ones(1/512)^T @ T_j  -> SBUF row (f32: the top-k
                # selection is precision-sensitive near ties) ----
                s_sb = pool.tile([1, 8 * 256], F32, tag="s_sb")
                for j2 in range(4):   # two j-blocks per 512-wide matmul
                    pss = pq.tile([1, 512], F32, tag="ps")
                    nc.tensor.matmul(pss[:], ones_sb[:],
                                     t16[:, 512 * j2:512 * j2 + 512],
                                     start=True, stop=True)
                    nc.scalar.activation(s_sb[:, 512 * j2:512 * j2 + 512],
                                         pss[:], AF.Identity)

                # ---- assemble r[128j+d] = sv_j[128+d] + sv_{j+1}[d] ----
                # (r32 shares the bmat slot: it is dead at max_index, before
                # this unit's bmat build; bmat(b) is dead once combine(b)'s
                # PE reads drain, well before r32(b+1) is written)
                r32 = pool.tile([1, L], F32, tag="bmat", name="r32")
                src_a = bass.AP(tensor=s_sb[:].tensor, offset=s_sb[:].offset + 128,
                                ap=[[s_sb[:].ap[0][0], 1], [256, 8], [1, 128]])
                nc.vector.tensor_copy(
                    r32[:].rearrange("p (j t) -> p j t", j=8), src_a)
                # += sv_{j+1}[d] for d>=1 ; j=0..6
                dst_b = bass.AP(tensor=r32[:].tensor, offset=r32[:].offset + 1,
                                ap=[[r32[:].ap[0][0], 1], [128, 7], [1, 127]])
                src_b = bass.AP(tensor=s_sb[:].tensor, offset=s_sb[:].offset + 257,
                                ap=[[s_sb[:].ap[0][0], 1], [256, 7], [1, 127]])
                nc.vector.tensor_tensor(dst_b, dst_b, src_b, ALU.add)
                # j=7 wraps to sv_0
                nc.vector.tensor_tensor(r32[:, 897:1024], r32[:, 897:1024],
                                        s_sb[:, 1:128], ALU.add)

                # ---- top-8 + softmax over top-6 ----
                top8 = pool.tile([1, 8], F32, tag="top8")
                idx8 = pool.tile([1, 8], DT.uint32, tag="idx8")
                nc.vector.max(top8[:], r32[:])
                nc.vector.max_index(idx8[:], top8[:], r32[:])
                negmax = pool.tile([1, 1], F32, tag="negmax")
                nc.vector.tensor_scalar_mul(negmax[:], top8[:, 0:1], -1.0)
                e6 = pool.tile([1, 8], F32, tag="e6")
                nc.scalar.activation(e6[:, 0:TOPK], top8[:, 0:TOPK], AF.Exp,
                                     bias=negmax[:], scale=1.0)
                se = pool.tile([1, 1], F32, tag="se")
                nc.vector.tensor_reduce(se[:], e6[:, 0:TOPK], AX.X, ALU.add)
                rse = pool.tile([1, 1], F32, tag="rse")
                nc.vector.reciprocal(rse[:], se[:])
                tc6 = pool.tile([1, 8], F32, tag="tc6")
                nc.vector.tensor_scalar_mul(tc6[:, 0:TOPK], e6[:, 0:TOPK], rse[:])
                # ---- broadcast tc + delays to all partitions via a rank-1
                # ones-row matmul (psum[p, j] = 1 * bcrow[0, j]) ----
                bcrow = pool.tile([1, 16], F32, tag="tcb", bufs=2)
                nc.vector.tensor_copy(bcrow[:, 0:TOPK], tc6[:, 0:TOPK])
                nc.vector.tensor_copy(bcrow[:, 8:8 + TOPK],
                                      idx8[0:1, 0:TOPK].bitcast(DT.int32))
                return bcrow

            def stage_b_chain2(b, bcrow):
                psbc = pq.tile([128, 16], F32, tag="ps")
                nc.tensor.matmul(psbc[:], onesr_sb[:], bcrow[:],
                                 start=True, stop=True)
                bc32 = pool.tile([128, 16], F32, tag="db8", bufs=2)
                nc.scalar.activation(bc32[:], psbc[:], AF.Identity)

                # ---- block-circulant shift matrix, 8 blocks of 128x128:
                # B[u, 128g+v] = sum_k tc_k * [(128g+u-v) mod 1024 == d_k]
                # (UG holds the mod-1024 iota; integers exact in f16) ----
                bmat = pool.tile([128, 1024], F16, tag="bmat", name="bmat")
                bsc = pool.tile([128, 1024], F16, tag="bsc", name="bsc")
                for k in range(TOPK):
                    dst = bmat if k == 0 else bsc
                    nc.vector.tensor_scalar(dst[:], ug_sb[:],
                                            bc32[:, 8 + k:9 + k],
                                            bc32[:, k:k + 1],
                                            ALU.is_equal, ALU.mult)
                    if k:
                        nc.vector.tensor_tensor(bmat[:], bmat[:], bsc[:],
                                                ALU.add)
                return bmat

            def stage_b_combine(b, vot, bmat):
                # ---- combine: h[c, 128T+v] += sum_g voT[(T+g)%8][u, c]^T
                # @ B_g[u, v], accumulated over g in PSUM ----
                for m in range(4):
                    for tb4 in range(2):
                        # four T-blocks share one psum bank (four accumulation
                        # groups in disjoint column quarters) so the h-update
                        # is one 512-wide DVE op instead of four 128-wide
                        psc = pq.tile([128, 512], F32, tag="ps")
                        for qtr in range(4):
                            tb = 4 * tb4 + qtr
                            for g in range(8):
                                j = (tb + g) % 8
                                nc.tensor.matmul(
                                    psc[:, 128 * qtr:128 * qtr + 128],
                                    vot[:, 512 * j + 128 * m:
                                        512 * j + 128 * m + 128],
                                    bmat[:, 128 * g:128 * g + 128],
                                    start=(g == 0), stop=(g == 7))
                        col = L * m + 512 * tb4
                        nc.vector.tensor_tensor(h_b[b][:, col:col + 512],
                                                h_b[b][:, col:col + 512],
                                                psc[:], ALU.add)

            def stage_b_back(l, b, ffn_sb):
                decomp(b)

                # ================= FFN =================
                # h16f reuses the h16q slot: h16q(b+1), cast in the pipelined
                # stage_a(b+1) emitted just before this, is dead once b+1's
                # QKV matmuls (earlier in the PE stream) have read it
                h16f = cast_h16(b, "h16q")
                for t in range(NTC):
                    # FFN intermediate lives in the dead q16/k16 slots (their
                    # tiles are consumed by stage_a's Gram, which for batch
                    # b+1 runs earlier in the PE stream than this FFN)
                    g16a = pool.tile([128, 8 * 512], F16, tag="q16",
                                     name="g16a")
                    g16b = pool.tile([128, 8 * 512], F16, tag="k16",
                                     name="g16b")
                    ghalf = [g16a, g16b]
                    for dm in range(16):
                        ps = pf.tile([128, 512], F32, tag="psf")
                        for k in range(4):
                            nc.tensor.matmul(
                                ps[:],
                                ffn_sb[:, 2048 * k + 128 * dm:
                                       2048 * k + 128 * dm + 128],
                                h16f[:, L * k + 512 * t: L * k + 512 * t + 512],
                                start=(k == 0), stop=(k == 3))
                        nc.scalar.activation(
                            ghalf[dm // 8][:, 512 * (dm % 8):
                                           512 * (dm % 8) + 512],
                            ps[:], AF.Gelu)
                    for m in range(4):
                        psy = pf.tile([128, 512], F32, tag="psf")
                        for k in range(16):
                            nc.tensor.matmul(
                                psy[:],
                                ffn_sb[:, 8192 + 512 * k + 128 * m:
                                       8192 + 512 * k + 128 * m + 128],
                                ghalf[k // 8][:, 512 * (k % 8):
                                              512 * (k % 8) + 512],
                                start=(k == 0), stop=(k == 15))
                        col = L * m + 512 * t
                        nc.vector.tensor_tensor(h_b[b][:, col:col + 512],
                                                h_b[b][:, col:col + 512],
                                                psy[:], ALU.add)
                decomp(b)

            def stage_final_stats(b):
                # ======== final my_Layernorm + projection for batch b ======
                hf = cast_h16(b, "h16q")
                # mu, sumsq rows via ones-matmuls (scaled by 1/512). stat
                # shares the cacc slot and mu2/sd the r32 slot: both tags'
                # pipelined uses (A-unit Gram / front r-assembly) interleave
                # in emission order with no cross-engine stalls.
                stat = pool.tile([1, 2 * L], F32, tag="cacc", name="stat")
                h2 = pool.tile([128, 4 * L], F16, tag="q16", name="h2")
                for m in range(4):
                    nc.scalar.activation(h2[:, L * m:L * m + L],
                                         hf[:, L * m:L * m + L], AF.Square)
                for which, src in ((0, hf), (1, h2)):
                    for t in range(NTC):
                        ps = pq.tile([1, 512], F32, tag="ps")
                        for k in range(4):
                            nc.tensor.matmul(
                                ps[:], ones_sb[:],
                                src[:, L * k + 512 * t: L * k + 512 * t + 512],
                                start=(k == 0), stop=(k == 3))
                        nc.vector.tensor_copy(
                            stat[:, which * L + 512 * t:
                                 which * L + 512 * t + 512], ps[:])
                # The projection is factored so its GEMMs depend only on hf,
                # never on a z-normalized tensor:
                #   out = P - mean_t(P) + pb,
                #   P[t,co] = inv[t]*A[t,co] - (inv*mu)[t]*colsum(pw)[co],
                #   A = hf^T @ pw.
                # The raw mu/ssq rows go time-major via a DRAM round trip;
                # the var/sqrt/recip/w math happens AFTER the transpose on
                # [128, 8] tiles (128 lanes) instead of 1-lane [1, 1024] rows.
                wr = nc.sync.dma_start(
                    dfin.ap()[b % 2].rearrange("r t -> (r t)"), stat[:])
                for prd in fin_reads.get(b % 2, []):
                    _add_dep_helper(wr.ins, prd.ins, sync=True,
                                    reason="fin WAR")
                # A[t, co] blocks: stationary = hf time-slices (ready at the
                # cast -- no wait on any normalization)
                a32 = pool.tile([128, 8 * 64], F32, tag="g16", name="a32")
                for tt in range(8):
                    ps = pf.tile([128, 512], F32, tag="psf")
                    for k in range(4):
                        nc.tensor.matmul(
                            ps[:, 0:64],
                            hf[:, L * k + 128 * tt: L * k + 128 * tt + 128],
                            projw_sb[:, 64 * k:64 * k + 64],
                            start=(k == 0), stop=(k == 3))
                    nc.scalar.activation(a32[:, 64 * tt:64 * tt + 64],
                                         ps[:, 0:64], AF.Identity)
                return a32, wr

            def stage_final_proj(b, a32, wr):
                # P assembly + time-mean subtraction + output store; emitted
                # one unit later so the DRAM round trip has fully landed
                invw = pool.tile([128, 16], F32, tag="db8", bufs=2,
                                 name="invw")
                for r in range(2):   # cols 0:8 = mu_t, 8:16 = ssq_t
                    rdap = bass.AP(tensor=dfin.ap().tensor,
                                   offset=2048 * (b % 2) + 1024 * r,
                                   ap=[[1, 128], [128, 8]])
                    rd = nc.sync.dma_start(invw[:, 8 * r:8 * r + 8], rdap)
                    _add_dep_helper(rd.ins, wr.ins, sync=True,
                                    reason="fin RAW")
                    fin_reads.setdefault(b % 2, []).append(rd)
                # inv = 1/sqrt(ssq - mu^2 + eps); w = inv*mu  (all [128, 8])
                iwt = pool.tile([128, 16], F32, tag="mz", name="iwt")
                nc.vector.tensor_tensor(iwt[:, 0:8], invw[:, 0:8],
                                        invw[:, 0:8], ALU.mult)
                nc.vector.tensor_tensor(iwt[:, 0:8], invw[:, 8:16],
                                        iwt[:, 0:8], ALU.subtract)
                nc.vector.tensor_scalar_add(iwt[:, 0:8], iwt[:, 0:8], EPS)
                nc.scalar.activation(iwt[:, 0:8], iwt[:, 0:8], AF.Sqrt)
                nc.vector.reciprocal(iwt[:, 0:8], iwt[:, 0:8])
                nc.vector.tensor_tensor(iwt[:, 8:16], iwt[:, 0:8],
                                        invw[:, 0:8], ALU.mult)
                for tt in range(8):
                    blk = a32[:, 64 * tt:64 * tt + 64]
                    nc.vector.tensor_scalar(blk, blk, iwt[:, tt:tt + 1],
                                            None, ALU.mult)
                    nc.vector.scalar_tensor_tensor(
                        blk, spwn_sb[:], iwt[:, 8 + tt:9 + tt], blk,
                        ALU.mult, ALU.add)
                # mean_t(P) row via ones(1/1024) matmuls, then pb - mean
                psm = pq.tile([1, 64], F32, tag="ps", padded_shape=[1, 512])
                for tt in range(8):
                    nc.tensor.matmul(psm[:], onesk_sb[:],
                                     a32[:, 64 * tt:64 * tt + 64],
                                     start=(tt == 0), stop=(tt == 7))
                pbm = pool.tile([1, 64], F32, tag="tcb", bufs=2, name="pbm")
                nc.vector.tensor_tensor(pbm[:], pbt_sb[0:1, :], psm[:],
                                        ALU.subtract)
                psb = pq.tile([128, 64], F32, tag="ps")
                nc.tensor.matmul(psb[:], onesr_sb[:], pbm[:],
                                 start=True, stop=True)
                o16 = pool.tile([128, 8 * 64], F16, tag="o32", name="o16")
                for tt in range(8):
                    nc.vector.tensor_tensor(o16[:, 64 * tt:64 * tt + 64],
                                            a32[:, 64 * tt:64 * tt + 64],
                                            psb[:], ALU.add)
                # one 3-dim DMA: out[b, 128*tt + p, co] <- o16[p, 64*tt + co]
                odst = bass.AP(tensor=out_e.ap().tensor, offset=b * L * 64,
                               ap=[[64, 128], [8192, 8], [1, 64]])
                nc.sync.dma_start(odst,
                                  o16[:].rearrange("p (t c) -> p t c", t=8))

            # -- software pipeline over ALL (l, b) units, 2 units deep: the
            # PE stream per unit is [combine(u)][QKV/Gram/voT(u+2)][FFN(u)],
            # so u's correlation chain (DMA/DVE/ACT) hides under u+1/u+2 PE
            # work, u's decomp latency hides under u+2's QKV/Gram, and layer
            # boundaries don't drain the pipeline (weights for l+1 load via
            # the gpsimd DMA queue two units ahead). The final my_Layernorm +
            # projection for batch b rides the tail of the last layer's unit.
            units = [(l, b) for l in range(nlayers) for b in range(nbatch)]
            vots, wmap = {}, {}

            def do_a(u):
                l, b = units[u]
                if b == 0:
                    wmap[l] = [load_qkvo(l), None]
                vots[u] = stage_a(l, b, wmap[l][0])

            # chain(u+1) is emitted a full unit before combine(u+1) so its
            # small-op DVE tail (r-assembly, top-8, softmax, B-build) runs
            # ahead of unit u's decomp work in the DVE queue and the combine
            # matmuls never wait on it; the shear-read DMA also stays ahead
            # of unit u+2's shear-writes in the Sync queue.
            do_a(0)
            if len(units) > 1:
                do_a(1)
            # finals are double-deferred: stats(b) runs after combine(u+1)
            # (so its matmuls never wait on decomp2(u)-gated casts ahead of
            # the combine), proj(b) after combine(u+2)
            bmats = {0: stage_b_chain2(0, stage_b_chain(0))}
            pending_stats = None
            pending_proj = None
            for u, (l, b) in enumerate(units):
                if b == 0:
                    wmap[l][1] = load_ffn(l)
                stage_b_combine(b, vots.pop(u), bmats.pop(u))
                if pending_proj is not None:
                    stage_final_proj(*pending_proj)
                    pending_proj = None
                if pending_stats is not None:
                    pending_proj = ((pending_stats,)
                                    + stage_final_stats(pending_stats))
                    pending_stats = None
                bcrow_n = (stage_b_chain(units[u + 1][1])
                           if u + 1 < len(units) else None)
                if u + 2 < len(units):
                    do_a(u + 2)
                if u + 1 < len(units):
                    bmats[u + 1] = stage_b_chain2(units[u + 1][1], bcrow_n)
                stage_b_back(l, b, wmap[l][1])
                if l == nlayers - 1:
                    pending_stats = b
            stage_final_proj(*pending_proj)
            stage_final_proj(pending_stats, *stage_final_stats(pending_stats))

    nc.compile()
    return nc


def _get_program(nbatch=4, nlayers=NL_TOT):
    key = (nbatch, nlayers)
    if key not in _BUILD_CACHE:
        _BUILD_CACHE[key] = _build(nbatch, nlayers)
    return _BUILD_CACHE[key]


def _prep_shared(inputs, nlayers):
    """Host-side input marshalling shared by all cores (weight layout/cast)."""
    f16 = np.float16
    wqkvo = np.stack([np.stack([np.asarray(inputs[n][l]).reshape(4, 128, 512)
                                for n in ("wq", "wk", "wv", "wo")])
                      for l in range(nlayers)]).astype(f16)
    w1 = np.stack([np.asarray(inputs["w1"][l]).reshape(4, 128, DFF)
                   for l in range(nlayers)]).astype(f16)
    w2 = np.stack([np.asarray(inputs["w2"][l]).reshape(16, 128, 512)
                   for l in range(nlayers)]).astype(f16)
    ew = np.asarray(inputs["embed_w"]).astype(f16)
    biases = np.zeros((128, nlayers * 16), np.float32)
    for l in range(nlayers):
        for w, n in enumerate(("bq", "bk", "bv", "bo")):
            arr = np.asarray(inputs[n][l])
            for m in range(4):
                biases[:, l * 16 + w * 4 + m] = arr[m * 128:(m + 1) * 128]
    pw_full = (np.asarray(inputs["ln_g"])[:, None]
               * np.asarray(inputs["proj_w"])).astype(np.float32)
    projw = pw_full.reshape(4, 128, 64).astype(f16)
    pbt = np.tile(np.asarray(inputs["proj_b"])[None, :], (128, 1)).astype(np.float32)
    # negated column sums of the (f16-rounded) ln_g-scaled projection, for
    # the factored final-LN: P = inv*A - (inv*mu)*colsum(pw)
    spwn = np.tile(-pw_full.astype(f16).astype(np.float32).sum(0)[None, :],
                   (128, 1)).astype(np.float32)
    onesk = np.full((128, 1), 1.0 / 1024, np.float32)
    # padded-cumsum edge ramps: front cs[i] = (i-12)*x0, back = total+k*xlast
    rampf = np.tile(np.arange(-PAD, 1, dtype=np.float32)[None, :], (128, 1))
    rampb = np.tile(np.arange(1, PAD + 1, dtype=np.float32)[None, :], (128, 1))
    ones512 = np.full((128, 1), 1.0 / 512, f16)
    # mod-1024 iota for the circulant shift blocks:
    # UG[u, 128g+v] = (128g + u - v) mod 1024 (integers <= 1023, f16-exact)
    u = np.arange(128)[:, None]
    v = np.arange(128)[None, :]
    ug = np.concatenate([(128 * g + u - v) % 1024 for g in range(8)],
                        axis=1).astype(f16)
    onesr = np.ones((1, 128), np.float32)
    return dict(wqkvo=wqkvo, w1=w1, w2=w2, ew=ew, biases=biases, projw=projw,
                pbt=pbt, rampf=rampf, rampb=rampb, ones512=ones512,
                ug=ug, onesr=onesr, spwn=spwn, onesk=onesk)


def _prep_x(xb):
    """(nb, L, CIN) fp32 -> (nb, 64, L+2) fp16 feature-major, circular padded."""
    xt = np.transpose(np.asarray(xb), (0, 2, 1))  # (nb, C, L)
    xe = np.concatenate([xt[:, :, -1:], xt, xt[:, :, :1]], axis=2)
    return xe.astype(np.float16)


def kernel(**inputs):
    from concourse.bass_utils import run_bass_kernel_spmd
    x = np.asarray(inputs["x"])
    B = x.shape[0]
    nbatch = B // NCORES
    nc = _get_program(nbatch, NL_TOT)
    shared = _prep_shared(inputs, NL_TOT)
    in_maps = []
    for c in range(NCORES):
        m = dict(shared)
        m["x"] = _prep_x(x[c * nbatch:(c + 1) * nbatch])
        in_maps.append(m)
    res = run_bass_kernel_spmd(nc, in_maps, core_ids=list(range(NCORES)))
    out = np.concatenate([res.results[c]["out"] for c in range(NCORES)], axis=0)
    return out.astype(np.float32)



# revision 110
# speedup vs baseline: 1.0176x; 1.0176x over previous
"""Autoformer encoder (nn_Autoformer_11441792876586) on 8 TRN2 NeuronCores.

Strategy: data-parallel over batch (4 batches/core). Per core, everything runs
in feature-major layout (channels on partitions, time on free dim):
  - all GEMMs: W stationary (din,dout)-tiles, activations moving -> fp16 in,
    fp32 PSUM accumulate
  - AutoCorrelation mean_corr via Gram matrix M' = K^T Q (c-contraction),
    block-diagonal sums C'_j accumulated straight from PSUM, DRAM shear read
    with ASCENDING element stride, ones-vector matmul partition reduction;
    top-8 via DVE Max8/MaxIndex; softmax on ACT/DVE
  - the weighted time-rolls sum_k tc_k*roll(V@Wo, d_k) run on the PE as a
    block-circulant matmul: V@Wo is produced TIME-major (voT, stationary =
    v16 time-slices), the 8 distinct 128x128 circulant blocks B_g[u,v] =
    sum_k tc_k*[(128g+u-v) mod 1024 == d_k] are built with one DVE
    is_equal+mult tensor_scalar per k against a host iota constant UG, and
    out[c, 128T+v] = sum_g voT[(T+g)%8]^T B_g accumulates in PSUM. This
    keeps gpsimd (Q7 ~27us/call dispatch stalls) entirely out of the kernel.
    bo is dropped: decomp(h + a + const) == decomp(h + a) since the
    edge-replicated moving average maps time-constants to themselves and
    softmax(tc) sums to 1.
  - all partition broadcasts via rank-1 ones-row matmuls (no gpsimd)
  - series_decomp via DVE prefix-scan cumsum + edge-replication corrections
  - residual stream h kept in f32 (the top-k selection has near-ties at
    ~1e-4 relative; an f16 stream drifts enough to flip them), cast to f16
    tiles right before each GEMM phase on the scalar engine
  - final my_Layernorm factored so the projection GEMMs depend only on the
    h cast, never a z-normalized tensor: out = P - mean_t(P) + pb with
    P = inv*(h^T pw) - (inv*mu)*colsum(pw); the inv/mu rows reach
    time-major per-partition form via a small DRAM round trip
Scheduling: all (layer, batch) units are software-pipelined 2 deep with
engines' in-order streams in mind. Per unit the emission order is
[combine(u)] [final-proj tail] [chain1(u+1): shear-read + s_j + top-8 +
softmax] [stage_a(u+2): QKV/Gram/voT + weight DMAs on the gpsimd queue]
[chain2(u+1): broadcast + B-build] [back(u): decomp/FFN/decomp], so the
PE always has the next unit's GEMMs queued while a unit's correlation
chain (DMA round trip + DVE small ops) resolves, and the B-build lands
in the DVE queue a full unit before its combine. gpsimd is used ONLY to
issue weight-load DMA descriptors: every Q7 compute op measured 3-27us
of dispatch latency and repeatedly regressed the span.
"""
import sys
if "/opt/trn_rl_repo" not in sys.path:
    sys.path.insert(0, "/opt/trn_rl_repo")
import hashlib as _hashlib
import os as _os
import numpy as np

# The neuronxcc NEFF cache keys on the HLO module hash, which does NOT cover
# the embedded bass program — a stale cache silently serves NEFFs built from
# an older kernel revision. Pin the cache to a dir derived from this file's
# contents so any source change forces a recompile.
_os.environ["NEURON_COMPILE_CACHE_URL"] = (
    "/tmp/neuron_cache_"
    + _hashlib.md5(open(__file__, "rb").read()).hexdigest()[:16])

L, CIN, D, DFF = 1024, 64, 512, 2048
NL_TOT, KMA, TOPK, EPS = 3, 25, 6, 1e-5
NCORES = 8
PAD = (KMA - 1) // 2  # 12
NTC = 2               # 512-wide time chunks per 1024
_BUILD_CACHE = {}


def _build(nbatch, nlayers):
    import concourse.bass as bass
    from concourse.bass import _add_dep_helper
    import concourse.bacc as bacc
    import concourse.tile as tile
    import concourse.mybir as mybir
    from contextlib import ExitStack

    DT = mybir.dt
    AF = mybir.ActivationFunctionType
    ALU = mybir.AluOpType
    AX = mybir.AxisListType
    F16, F32 = DT.float16, DT.float32

    nc = bacc.Bacc("TRN2", target_bir_lowering=False, debug=False,
                   num_devices=NCORES)

    # ---------------- I/O ----------------
    x_e = nc.dram_tensor("x", [nbatch, 64, L + 2], F16, kind="ExternalInput")
    ug_e = nc.dram_tensor("ug", [128, 1024], F16, kind="ExternalInput")
    onesr_e = nc.dram_tensor("onesr", [1, 128], F32, kind="ExternalInput")
    wqkvo_e = nc.dram_tensor("wqkvo", [nlayers, 4, 4, 128, 512], F16, kind="ExternalInput")
    w1_e = nc.dram_tensor("w1", [nlayers, 4, 128, DFF], F16, kind="ExternalInput")
    w2_e = nc.dram_tensor("w2", [nlayers, 16, 128, 512], F16, kind="ExternalInput")
    ew_e = nc.dram_tensor("ew", [3, 64, 512], F16, kind="ExternalInput")
    bias_e = nc.dram_tensor("biases", [128, nlayers * 16], F32, kind="ExternalInput")
    projw_e = nc.dram_tensor("projw", [4, 128, 64], F16, kind="ExternalInput")
    pbt_e = nc.dram_tensor("pbt", [128, 64], F32, kind="ExternalInput")
    spwn_e = nc.dram_tensor("spwn", [128, 64], F32, kind="ExternalInput")
    onesk_e = nc.dram_tensor("onesk", [128, 1], F32, kind="ExternalInput")
    # final-LN row transpose scratch (parity per batch)
    dfin = nc.dram_tensor("dfin", [2, 2, 1024], F32)
    rampf_e = nc.dram_tensor("rampf", [128, PAD + 1], F32, kind="ExternalInput")
    rampb_e = nc.dram_tensor("rampb", [128, PAD], F32, kind="ExternalInput")
    ones_e = nc.dram_tensor("ones512", [128, 1], F16, kind="ExternalInput")
    out_e = nc.dram_tensor("out", [nbatch, L, 64], F16, kind="ExternalOutput")
    # internal DRAM shear buffers (alternating parity per batch)
    dsh = [nc.dram_tensor(f"dsh{p}", [8, 128, 512], F16) for p in range(2)]

    with tile.TileContext(nc) as tc, ExitStack() as ctx:
        pool = ctx.enter_context(tc.tile_pool(name="sb", bufs=1))
        wpool = ctx.enter_context(tc.tile_pool(name="wp", bufs=1))
        pq = ctx.enter_context(tc.tile_pool(name="pq", bufs=5, space="PSUM"))
        pf = ctx.enter_context(tc.tile_pool(name="pf", bufs=3, space="PSUM"))

        # ------------- persistent constants -------------
        # ug + ew first: ug feeds the PE warm-up immediately and ew feeds
        # the embed; the rest can land while those run
        ug_sb = pool.tile([128, 1024], F16, tag="ug")
        nc.sync.dma_start(ug_sb[:], ug_e.ap())
        # PE clock warm-up: ~5us of dummy matmuls overlapping the remaining
        # constant/input DMAs, so the embed and first QKV stream at 2.4GHz
        # instead of the 1.2GHz cold clock (PE ramps after ~4us sustained)
        warm = pool.tile([128, 512], F16, tag="cw16", name="warm")
        for wi in range(12):
            psw = pq.tile([128, 512], F32, tag="ps")
            nc.tensor.matmul(psw[:], ug_sb[:, 0:128], ug_sb[:, 0:512],
                             start=True, stop=True)
            if wi == 11:
                nc.scalar.activation(warm[:], psw[:], AF.Identity)
        # ew lives in the g16 slot: it is only read during the embed stage,
        # before the first FFN allocates g16
        ew_sb = pool.tile([64, 3 * 512], F16, tag="g16", name="ew_sb")
        nc.sync.dma_start(ew_sb[:].rearrange("p (j c) -> p j c", j=3),
                          ew_e.ap().rearrange("j p c -> p j c"))
        # batch 0's input ahead of the lower-priority constants: the first
        # embed matmul otherwise idles ~16us while x(0) queues behind them
        x0_sb = pool.tile([64, L + 2], F16, tag="cs", name="x_sb")
        nc.sync.dma_start(x0_sb[:], x_e.ap()[0])
        bias_sb = pool.tile([128, nlayers * 16], F32, tag="bias")
        nc.sync.dma_start(bias_sb[:], bias_e.ap())
        projw_sb = pool.tile([128, 4 * 64], F16, tag="projw")
        nc.sync.dma_start(projw_sb[:].rearrange("p (k c) -> p k c", k=4),
                          projw_e.ap().rearrange("k p c -> p k c"))
        pbt_sb = pool.tile([128, 64], F32, tag="pbt")
        nc.sync.dma_start(pbt_sb[:], pbt_e.ap())
        spwn_sb = pool.tile([128, 64], F32, tag="spwn")
        nc.sync.dma_start(spwn_sb[:], spwn_e.ap())
        onesk_sb = pool.tile([128, 1], F32, tag="onesk")
        nc.sync.dma_start(onesk_sb[:], onesk_e.ap())
        rampf_sb = pool.tile([128, PAD + 1], F32, tag="rampf")
        nc.sync.dma_start(rampf_sb[:], rampf_e.ap())
        rampb_sb = pool.tile([128, PAD], F32, tag="rampb")
        nc.sync.dma_start(rampb_sb[:], rampb_e.ap())
        ones_sb = pool.tile([128, 1], F16, tag="ones")
        nc.sync.dma_start(ones_sb[:], ones_e.ap())
        onesr_sb = pool.tile([1, 128], F32, tag="onesr")
        nc.sync.dma_start(onesr_sb[:], onesr_e.ap())

        zero_writes = {0: [], 1: []}
        shear_writes = [{}, {}]
        shear_reads = [{}, {}]
        fin_reads = {}

        # persistent per-batch h (f32, feature-major: c-tile m at cols [1024m))
        h_b = [pool.tile([128, 4 * L], F32, tag=f"h{b}", name=f"h{b}")
               for b in range(nbatch)]

        def cast_h16(b, tag):
            """f16 snapshot of h for GEMM consumption, on the scalar engine
            (ACT Identity rounds f32->f16 the same as a DVE copy, and the
            scalar engine has headroom while DVE is the bottleneck)."""
            h16 = pool.tile([128, 4 * L], F16, tag=tag, name="h16")
            for m in range(4):
                nc.scalar.activation(h16[:, m * L:(m + 1) * L],
                                     h_b[b][:, m * L:(m + 1) * L], AF.Identity)
            return h16

        def gemm_512(dst_sb, dst_col, w_sb, w_base, rhs_sb, psum_pool,
                     bias_ap=None, act=None, nk=4):
            """dst[:, dst_col + m*L + tc*512] = act(sum_k W[k,m]^T @ rhs[k,tc]) + bias
            W blocks at w_sb[:, w_base + 512k + 128m]; rhs c-tile k at rhs_sb cols
            [L*k], time chunk tc at [512tc]. dst layout: c-tile m at [L*m]."""
            for m in range(4):
                for t in range(NTC):
                    ps = psum_pool.tile([128, 512], F32, tag="ps")
                    for k in range(nk):
                        nc.tensor.matmul(
                            ps[:],
                            w_sb[:, w_base + 512 * k + 128 * m:
                                 w_base + 512 * k + 128 * m + 128],
                            rhs_sb[:, L * k + 512 * t: L * k + 512 * t + 512],
                            start=(k == 0), stop=(k == nk - 1))
                    col = dst_col + L * m + 512 * t
                    nc.scalar.activation(dst_sb[:, col:col + 512], ps[:],
                                         act or AF.Identity,
                                         bias=bias_ap[m] if bias_ap else 0.0)

        def bias_aps(l, w):
            return [bias_sb[:, l * 16 + w * 4 + m: l * 16 + w * 4 + m + 1]
                    for m in range(4)]

        def decomp(b):
            """h <- h - moving_average(h) with edge replication; h=(128,4L) f32.

            The cs tile holds the EXACT padded cumsum (cs[i] = sum of the
            first i entries of the edge-replicated sequence, up to a global
            constant that cancels in the windowed diff): front pad =
            (i-12)*x0 via one tensor_scalar on an iota constant, scan with
            zero initial, back pad = total + k*x_last via one STT. No
            separate edge corrections or saved edge columns needed.
            DVE, not gpsimd: decomp is on the critical path and the Q7
            handoff adds ~2-4us latency per call."""
            h = h_b[b]
            for m in range(4):
                hx = h[:, m * L:(m + 1) * L]
                cs = pool.tile([128, L + 2 * PAD + 4], F32, tag="cs",
                               name="cs")
                nc.vector.tensor_scalar(cs[:, 0:PAD + 1], rampf_sb[:],
                                        hx[:, 0:1], None, ALU.mult)
                nc.vector.tensor_tensor_scan(cs[:, PAD + 1:PAD + 1 + L], hx, hx,
                                             0.0, ALU.add, ALU.bypass)
                nc.vector.scalar_tensor_tensor(
                    cs[:, PAD + 1 + L:PAD + 1 + L + PAD], rampb_sb[:],
                    hx[:, L - 1:L],
                    cs[:, PAD + L:PAD + 1 + L].to_broadcast((128, PAD)),
                    ALU.mult, ALU.add)
                # windowed sum A[t] = cs[t+25] - cs[t]; h = hx - A/25
                tmp = pool.tile([128, L], F32, tag="tmp", name="tmp")
                nc.vector.tensor_tensor(tmp[:], cs[:, KMA:KMA + L], cs[:, 0:L],
                                        ALU.subtract)
                nc.vector.scalar_tensor_tensor(hx, tmp[:], -1.0 / KMA, hx,
                                               ALU.mult, ALU.add)

        # ================= embed =================
        for b in range(nbatch):
            if b == 0:
                x_sb = x0_sb
            else:
                x_sb = pool.tile([64, L + 2], F16, tag="cs", name="x_sb")
                nc.sync.dma_start(x_sb[:], x_e.ap()[b])
            for m in range(4):
                for t in range(NTC):
                    ps = pq.tile([128, 512], F32, tag="ps")
                    for j in range(3):
                        nc.tensor.matmul(
                            ps[:],
                            ew_sb[0:64, 512 * j + 128 * m: 512 * j + 128 * m + 128],
                            x_sb[0:64, j + 512 * t: j + 512 * t + 512],
                            start=(j == 0), stop=(j == 2))
                    nc.scalar.activation(
                        h_b[b][:, L * m + 512 * t: L * m + 512 * t + 512],
                        ps[:], AF.Identity)

        # zero the shear scratch in DRAM once; emitted AFTER the embed so
        # these 2MB of writes queue behind the x loads the head needs first
        # (they only have to precede the first shear read, ~100us in)
        zero_sb = pool.tile([128, 512], F16, tag="cs", name="zero_sb")
        nc.vector.memset(zero_sb[:], 0.0)
        for p in range(2):
            for j in range(8):
                zero_writes[p].append(
                    nc.sync.dma_start(dsh[p].ap()[j], zero_sb[:]))

        # ================= layers (flat (l, b) unit pipeline) =================
        def load_qkvo(l):
            """Weight loads ride the GpSimd DMA queue (not Sync), so a load
            whose WAR-wait on the previous layer's last reads hasn't cleared
            can't head-of-line-block the Sync queue's shear DMAs. The qkvo
            load is emitted two units ahead (WAR on voT(l-1, last) clears
            early); the ffn load is emitted at unit (l, 0) itself since its
            WAR only clears at FFN2(l-1, last)."""
            qkvo_sb = wpool.tile([128, 4 * 2048], F16, tag="qkvo")
            nc.gpsimd.dma_start(
                qkvo_sb[:].rearrange("p (w k c) -> p w k c", w=4, k=4),
                wqkvo_e.ap()[l].rearrange("w k p c -> p w k c"))
            return qkvo_sb

        def load_ffn(l):
            ffn_sb = wpool.tile([128, 16384], F16, tag="ffn")
            nc.gpsimd.dma_start(
                ffn_sb[:, 0:8192].rearrange("p (k c) -> p k c", k=4),
                w1_e.ap()[l].rearrange("k p c -> p k c"))
            nc.gpsimd.dma_start(
                ffn_sb[:, 8192:16384].rearrange("p (k c) -> p k c", k=16),
                w2_e.ap()[l].rearrange("k p c -> p k c"))
            return ffn_sb

        if True:
            def stage_a(l, b, qkvo_sb):
                """PE-heavy front half: QKV gemms, Gram + shear write, voT.
                Emitted one batch AHEAD of stage_b(b-1) so the PE stream has
                queued work while b-1's correlation chain (DMA/DVE/ACT small
                ops) resolves."""
                h16 = cast_h16(b, "h16q")
                q16 = pool.tile([128, 4 * L], F16, tag="q16")
                k16 = pool.tile([128, 4 * L], F16, tag="k16")
                v16 = pool.tile([128, 4 * L], F16, tag="v16", name="v16")
                gemm_512(q16, 0, qkvo_sb, 0, h16, pq, bias_aps(l, 0))
                gemm_512(k16, 0, qkvo_sb, 2048, h16, pq, bias_aps(l, 1))
                gemm_512(v16, 0, qkvo_sb, 4096, h16, pq, bias_aps(l, 2))

                # ---- Gram M'_i = K_i^T Q (fp32 psum chunks, rounded to f16
                # per chunk); C'_j then summed from the f16 chunks on DVE:
                # C'_j[p,u] = sum_i M'_i[p, 128*((i+j)%8)+u]. A j-major
                # all-PSUM variant (32 128-wide matmuls per j) measured
                # +0.36ms: 128-wide moving exposes the 87ns LDWEIGHTS that a
                # 512-wide stream hides, so keep the 512-wide chunks.
                cacc = pool.tile([128, 1024], F32, tag="cacc")
                cw16 = pool.tile([128, 1024], F16, tag="cw16", name="cw16")
                for i in range(8):
                    for t2 in range(2):
                        psm = pq.tile([128, 512], F32, tag="ps")
                        for kc in range(4):
                            nc.tensor.matmul(
                                psm[:],
                                k16[:, L * kc + 128 * i: L * kc + 128 * i + 128],
                                q16[:, L * kc + 512 * t2: L * kc + 512 * t2 + 512],
                                start=(kc == 0), stop=(kc == 3))
                        chunk16 = pool.tile([128, 512], F16, tag="chunk16",
                                            bufs=2, name="chunk16")
                        nc.scalar.activation(chunk16[:], psm[:], AF.Identity)
                        for qb in range(4):
                            jq = 4 * t2 + qb          # q-time block in this chunk
                            j = (jq - i) % 8          # C'_j slice it feeds
                            blk = chunk16[:, 128 * qb:128 * qb + 128]
                            dst32 = cacc[:, 128 * j:128 * j + 128]
                            if i == 0:
                                nc.vector.tensor_copy(dst32, blk)
                            elif i < 7:
                                nc.vector.tensor_tensor(dst32, dst32, blk,
                                                        ALU.add)
                            else:
                                nc.vector.tensor_tensor(
                                    cw16[:, 128 * j:128 * j + 128], dst32, blk,
                                    ALU.add)
                # all 8 C'_j blocks in ONE 3-dim DMA (dst order (p, j, c))
                shw = bass.AP(tensor=dsh[b % 2].ap().tensor, offset=256,
                              ap=[[512, 128], [128 * 512, 8], [1, 128]])
                wr = nc.sync.dma_start(
                    shw, cw16[:].rearrange("p (j c) -> p j c", j=8))
                for j in range(8):
                    for prd in shear_reads[b % 2].get(j, []):
                        _add_dep_helper(wr.ins, prd.ins, sync=True,
                                        reason="shear WAR")
                    shear_writes[b % 2][j] = wr

                # voT = (v @ wo) TIME-major: voT_T[u, c] = sum_cin
                # v16[cin, 128T+u] * wo[cin, c]; stationary = v16 time-slices,
                # moving = wo [cin, 512 cout] (contiguous in qkvo_sb). bo is
                # dropped (decomp kills time-constant shifts, softmax sums
                # to 1). Emitted after the Gram so corr progress is preferred.
                vot = pool.tile([128, 8 * 512], F16, tag="vo2", bufs=2,
                                name="vot")
                for tb in range(8):
                    psv = pq.tile([128, 512], F32, tag="ps")
                    for kc in range(4):
                        nc.tensor.matmul(
                            psv[:],
                            v16[:, L * kc + 128 * tb: L * kc + 128 * tb + 128],
                            qkvo_sb[:, 6144 + 512 * kc: 6144 + 512 * kc + 512],
                            start=(kc == 0), stop=(kc == 3))
                    nc.scalar.activation(vot[:, 512 * tb:512 * tb + 512],
                                         psv[:], AF.Identity)
                return vot

            def stage_b_chain(b):
                # ---- shear read T[p, (j,u')] = D[j, p, p + 128 + u'] ----
                # ascending inner stride: 256 contiguous f16 = 512B/partition
                t16 = pool.tile([128, 8 * 256], F16, tag="t16", name="t16")
                shear_in = bass.AP(
                    tensor=dsh[b % 2].ap().tensor,
                    offset=128,
                    ap=[[513, 128], [128 * 512, 8], [1, 256]])
                rd = nc.sync.dma_start(
                    t16[:].rearrange("p (j t) -> p j t", j=8), shear_in)
                for j in range(8):
                    _add_dep_helper(rd.ins, shear_writes[b % 2][j].ins,
                                    sync=True, reason="shear RAW")
                    shear_reads[b % 2].setdefault(j, []).append(rd)
                for zw in zero_writes[b % 2]:
                    _add_dep_helper(rd.ins, zw.ins, sync=True,
                                    reason="shear zero RAW")

                # ---- s_j = ones(1/512)^T @ T_j  -> SBUF row (f32: the top-k
                # selection is precision-sensitive near ties) ----
                s_sb = pool.tile([1, 8 * 256], F32, tag="s_sb")
                for j2 in range(4):   # two j-blocks per 512-wide matmul
                    pss = pq.tile([1, 512], F32, tag="ps")
                    nc.tensor.matmul(pss[:], ones_sb[:],
                                     t16[:, 512 * j2:512 * j2 + 512],
                                     start=True, stop=True)
                    nc.scalar.activation(s_sb[:, 512 * j2:512 * j2 + 512],
                                         pss[:], AF.Identity)

                # ---- assemble r[128j+d] = sv_j[128+d] + sv_{j+1}[d] ----
                # (r32 shares the bmat slot: it is dead at max_index, before
                # this unit's bmat build; bmat(b) is dead once combine(b)'s
                # PE reads drain, well before r32(b+1) is written)
                r32 = pool.tile([1, L], F32, tag="bmat", name="r32")
                src_a = bass.AP(tensor=s_sb[:].tensor, offset=s_sb[:].offset + 128,
                                ap=[[s_sb[:].ap[0][0], 1], [256, 8], [1, 128]])
                nc.vector.tensor_copy(
                    r32[:].rearrange("p (j t) -> p j t", j=8), src_a)
                # += sv_{j+1}[d] for d>=1 ; j=0..6
                dst_b = bass.AP(tensor=r32[:].tensor, offset=r32[:].offset + 1,
                                ap=[[r32[:].ap[0][0], 1], [128, 7], [1, 127]])
                src_b = bass.AP(tensor=s_sb[:].tensor, offset=s_sb[:].offset + 257,
                                ap=[[s_sb[:].ap[0][0], 1], [256, 7], [1, 127]])
                nc.vector.tensor_tensor(dst_b, dst_b, src_b, ALU.add)
                # j=7 wraps to sv_0
                nc.vector.tensor_tensor(r32[:, 897:1024], r32[:, 897:1024],
                                        s_sb[:, 1:128], ALU.add)

                # ---- top-8 + softmax over top-6 ----
                top8 = pool.tile([1, 8], F32, tag="top8")
                idx8 = pool.tile([1, 8], DT.uint32, tag="idx8")
                nc.vector.max(top8[:], r32[:])
                nc.vector.max_index(idx8[:], top8[:], r32[:])
                negmax = pool.tile([1, 1], F32, tag="negmax")
                nc.vector.tensor_scalar_mul(negmax[:], top8[:, 0:1], -1.0)
                e6 = pool.tile([1, 8], F32, tag="e6")
                nc.scalar.activation(e6[:, 0:TOPK], top8[:, 0:TOPK], AF.Exp,
                                     bias=negmax[:], scale=1.0)
                se = pool.tile([1, 1], F32, tag="se")
                nc.vector.tensor_reduce(se[:], e6[:, 0:TOPK], AX.X, ALU.add)
                rse = pool.tile([1, 1], F32, tag="rse")
                nc.vector.reciprocal(rse[:], se[:])
                tc6 = pool.tile([1, 8], F32, tag="tc6")
                nc.vector.tensor_scalar_mul(tc6[:, 0:TOPK], e6[:, 0:TOPK], rse[:])
                # ---- broadcast tc + delays to all partitions via a rank-1
                # ones-row matmul (psum[p, j] = 1 * bcrow[0, j]) ----
                bcrow = pool.tile([1, 16], F32, tag="tcb", bufs=2)
                nc.vector.tensor_copy(bcrow[:, 0:TOPK], tc6[:, 0:TOPK])
                nc.vector.tensor_copy(bcrow[:, 8:8 + TOPK],
                                      idx8[0:1, 0:TOPK].bitcast(DT.int32))
                return bcrow

            def stage_b_chain2(b, bcrow):
                psbc = pq.tile([128, 16], F32, tag="ps")
                nc.tensor.matmul(psbc[:], onesr_sb[:], bcrow[:],
                                 start=True, stop=True)
                bc32 = pool.tile([128, 16], F32, tag="db8", bufs=2)
                nc.scalar.activation(bc32[:], psbc[:], AF.Identity)

                # ---- block-circulant shift matrix, 8 blocks of 128x128:
                # B[u, 128g+v] = sum_k tc_k * [(128g+u-v) mod 1024 == d_k]
                # (UG holds the mod-1024 iota; integers exact in f16) ----
                bmat = pool.tile([128, 1024], F16, tag="bmat", name="bmat")
                bsc = pool.tile([128, 1024], F16, tag="bsc", name="bsc")
                for k in range(TOPK):
                    dst = bmat if k == 0 else bsc
                    nc.vector.tensor_scalar(dst[:], ug_sb[:],
                                            bc32[:, 8 + k:9 + k],
                                            bc32[:, k:k + 1],
                                            ALU.is_equal, ALU.mult)
                    if k:
                        nc.vector.tensor_tensor(bmat[:], bmat[:], bsc[:],
                                                ALU.add)
                return bmat

            def stage_b_combine(b, vot, bmat):
                # ---- combine: h[c, 128T+v] += sum_g voT[(T+g)%8][u, c]^T
                # @ B_g[u, v], accumulated over g in PSUM ----
                for m in range(4):
                    for tb4 in range(2):
                        # four T-blocks share one psum bank (four accumulation
                        # groups in disjoint column quarters) so the h-update
                        # is one 512-wide DVE op instead of four 128-wide
                        psc = pq.tile([128, 512], F32, tag="ps")
                        for qtr in range(4):
                            tb = 4 * tb4 + qtr
                            for g in range(8):
                                j = (tb + g) % 8
                                nc.tensor.matmul(
                                    psc[:, 128 * qtr:128 * qtr + 128],
                                    vot[:, 512 * j + 128 * m:
                                        512 * j + 128 * m + 128],
                                    bmat[:, 128 * g:128 * g + 128],
                                    start=(g == 0), stop=(g == 7))
                        col = L * m + 512 * tb4
                        nc.vector.tensor_tensor(h_b[b][:, col:col + 512],
                                                h_b[b][:, col:col + 512],
                                                psc[:], ALU.add)

            def stage_b_back(l, b, ffn_sb):
                decomp(b)

                # ================= FFN =================
                # h16f reuses the h16q slot: h16q(b+1), cast in the pipelined
                # stage_a(b+1) emitted just before this, is dead once b+1's
                # QKV matmuls (earlier in the PE stream) have read it
                h16f = cast_h16(b, "h16q")
                for t in range(NTC):
                    # FFN intermediate lives in the dead q16/k16 slots (their
                    # tiles are consumed by stage_a's Gram, which for batch
                    # b+1 runs earlier in the PE stream than this FFN)
                    g16a = pool.tile([128, 8 * 512], F16, tag="q16",
                                     name="g16a")
                    g16b = pool.tile([128, 8 * 512], F16, tag="k16",
                                     name="g16b")
                    ghalf = [g16a, g16b]
                    for dm in range(16):
                        ps = pf.tile([128, 512], F32, tag="psf")
                        for k in range(4):
                            nc.tensor.matmul(
                                ps[:],
                                ffn_sb[:, 2048 * k + 128 * dm:
                                       2048 * k + 128 * dm + 128],
                                h16f[:, L * k + 512 * t: L * k + 512 * t + 512],
                                start=(k == 0), stop=(k == 3))
                        nc.scalar.activation(
                            ghalf[dm // 8][:, 512 * (dm % 8):
                                           512 * (dm % 8) + 512],
                            ps[:], AF.Gelu)
                    for m in range(4):
                        psy = pf.tile([128, 512], F32, tag="psf")
                        for k in range(16):
                            nc.tensor.matmul(
                                psy[:],
                                ffn_sb[:, 8192 + 512 * k + 128 * m:
                                       8192 + 512 * k + 128 * m + 128],
                                ghalf[k // 8][:, 512 * (k % 8):
                                              512 * (k % 8) + 512],
                                start=(k == 0), stop=(k == 15))
                        col = L * m + 512 * t
                        nc.vector.tensor_tensor(h_b[b][:, col:col + 512],
                                                h_b[b][:, col:col + 512],
                                                psy[:], ALU.add)
                decomp(b)

            def stage_final_stats(b):
                # ======== final my_Layernorm + projection for batch b ======
                hf = cast_h16(b, "h16q")
                # mu, sumsq rows via ones-matmuls (scaled by 1/512). stat
                # shares the cacc slot and mu2/sd the r32 slot: both tags'
                # pipelined uses (A-unit Gram / front r-assembly) interleave
                # in emission order with no cross-engine stalls.
                stat = pool.tile([1, 2 * L], F32, tag="cacc", name="stat")
                h2 = pool.tile([128, 4 * L], F16, tag="q16", name="h2")
                for m in range(4):
                    nc.scalar.activation(h2[:, L * m:L * m + L],
                                         hf[:, L * m:L * m + L], AF.Square)
                for which, src in ((0, hf), (1, h2)):
                    for t in range(NTC):
                        ps = pq.tile([1, 512], F32, tag="ps")
                        for k in range(4):
                            nc.tensor.matmul(
                                ps[:], ones_sb[:],
                                src[:, L * k + 512 * t: L * k + 512 * t + 512],
                                start=(k == 0), stop=(k == 3))
                        nc.vector.tensor_copy(
                            stat[:, which * L + 512 * t:
                                 which * L + 512 * t + 512], ps[:])
                # The projection is factored so its GEMMs depend only on hf,
                # never on a z-normalized tensor:
                #   out = P - mean_t(P) + pb,
                #   P[t,co] = inv[t]*A[t,co] - (inv*mu)[t]*colsum(pw)[co],
                #   A = hf^T @ pw.
                # The raw mu/ssq rows go time-major via a DRAM round trip;
                # the var/sqrt/recip/w math happens AFTER the transpose on
                # [128, 8] tiles (128 lanes) instead of 1-lane [1, 1024] rows.
                wr = nc.sync.dma_start(
                    dfin.ap()[b % 2].rearrange("r t -> (r t)"), stat[:])
                for prd in fin_reads.get(b % 2, []):
                    _add_dep_helper(wr.ins, prd.ins, sync=True,
                                    reason="fin WAR")
                # A[t, co] blocks: stationary = hf time-slices (ready at the
                # cast -- no wait on any normalization)
                a32 = pool.tile([128, 8 * 64], F32, tag="g16", name="a32")
                for tt in range(8):
                    ps = pf.tile([128, 512], F32, tag="psf")
                    for k in range(4):
                        nc.tensor.matmul(
                            ps[:, 0:64],
                            hf[:, L * k + 128 * tt: L * k + 128 * tt + 128],
                            projw_sb[:, 64 * k:64 * k + 64],
                            start=(k == 0), stop=(k == 3))
                    nc.scalar.activation(a32[:, 64 * tt:64 * tt + 64],
                                         ps[:, 0:64], AF.Identity)
                return a32, wr

            def stage_final_proj(b, a32, wr):
                # P assembly + time-mean subtraction + output store; emitted
                # one unit later so the DRAM round trip has fully landed
                invw = pool.tile([128, 16], F32, tag="db8", bufs=2,
                                 name="invw")
                for r in range(2):   # cols 0:8 = mu_t, 8:16 = ssq_t
                    rdap = bass.AP(tensor=dfin.ap().tensor,
                                   offset=2048 * (b % 2) + 1024 * r,
                                   ap=[[1, 128], [128, 8]])
                    rd = nc.sync.dma_start(invw[:, 8 * r:8 * r + 8], rdap)
                    _add_dep_helper(rd.ins, wr.ins, sync=True,
                                    reason="fin RAW")
                    fin_reads.setdefault(b % 2, []).append(rd)
                # inv = 1/sqrt(ssq - mu^2 + eps); w = inv*mu  (all [128, 8])
                iwt = pool.tile([128, 16], F32, tag="mz", name="iwt")
                nc.vector.tensor_tensor(iwt[:, 0:8], invw[:, 0:8],
                                        invw[:, 0:8], ALU.mult)
                nc.vector.tensor_tensor(iwt[:, 0:8], invw[:, 8:16],
                                        iwt[:, 0:8], ALU.subtract)
                nc.vector.tensor_scalar_add(iwt[:, 0:8], iwt[:, 0:8], EPS)
                nc.scalar.activation(iwt[:, 0:8], iwt[:, 0:8], AF.Sqrt)
                nc.vector.reciprocal(iwt[:, 0:8], iwt[:, 0:8])
                nc.vector.tensor_tensor(iwt[:, 8:16], iwt[:, 0:8],
                                        invw[:, 0:8], ALU.mult)
                for tt in range(8):
                    blk = a32[:, 64 * tt:64 * tt + 64]
                    nc.vector.tensor_scalar(blk, blk, iwt[:, tt:tt + 1],
                                            None, ALU.mult)
                    nc.vector.scalar_tensor_tensor(
                        blk, spwn_sb[:], iwt[:, 8 + tt:9 + tt], blk,
                        ALU.mult, ALU.add)
                # mean_t(P) row via ones(1/1024) matmuls, then pb - mean
                psm = pq.tile([1, 64], F32, tag="ps", padded_shape=[1, 512])
                for tt in range(8):
                    nc.tensor.matmul(psm[:], onesk_sb[:],
                                     a32[:, 64 * tt:64 * tt + 64],
                                     start=(tt == 0), stop=(tt == 7))
                pbm = pool.tile([1, 64], F32, tag="tcb", bufs=2, name="pbm")
                nc.vector.tensor_tensor(pbm[:], pbt_sb[0:1, :], psm[:],
                                        ALU.subtract)
                psb = pq.tile([128, 64], F32, tag="ps")
                nc.tensor.matmul(psb[:], onesr_sb[:], pbm[:],
                                 start=True, stop=True)
                o16 = pool.tile([128, 8 * 64], F16, tag="o32", name="o16")
                for tt in range(8):
                    nc.vector.tensor_tensor(o16[:, 64 * tt:64 * tt + 64],
                                            a32[:, 64 * tt:64 * tt + 64],
                                            psb[:], ALU.add)
                # one 3-dim DMA: out[b, 128*tt + p, co] <- o16[p, 64*tt + co]
                odst = bass.AP(tensor=out_e.ap().tensor, offset=b * L * 64,
                               ap=[[64, 128], [8192, 8], [1, 64]])
                nc.sync.dma_start(odst,
                                  o16[:].rearrange("p (t c) -> p t c", t=8))

            # -- software pipeline over ALL (l, b) units, 2 units deep: the
            # PE stream per unit is [combine(u)][QKV/Gram/voT(u+2)][FFN(u)],
            # so u's correlation chain (DMA/DVE/ACT) hides under u+1/u+2 PE
            # work, u's decomp latency hides under u+2's QKV/Gram, and layer
            # boundaries don't drain the pipeline (weights for l+1 load via
            # the gpsimd DMA queue two units ahead). The final my_Layernorm +
            # projection for batch b rides the tail of the last layer's unit.
            units = [(l, b) for l in range(nlayers) for b in range(nbatch)]
            vots, wmap = {}, {}

            def do_a(u):
                l, b = units[u]
                if b == 0:
                    wmap[l] = [load_qkvo(l), None]
                vots[u] = stage_a(l, b, wmap[l][0])

            # chain(u+1) is emitted a full unit before combine(u+1) so its
            # small-op DVE tail (r-assembly, top-8, softmax, B-build) runs
            # ahead of unit u's decomp work in the DVE queue and the combine
            # matmuls never wait on it; the shear-read DMA also stays ahead
            # of unit u+2's shear-writes in the Sync queue.
            do_a(0)
            if len(units) > 1:
                do_a(1)
            # finals are double-deferred: stats(b) runs after combine(u+1)
            # (so its matmuls never wait on decomp2(u)-gated casts ahead of
            # the combine), proj(b) after combine(u+2)
            bmats = {0: stage_b_chain2(0, stage_b_chain(0))}
            pending_stats = None
            pending_proj = None
            for u, (l, b) in enumerate(units):
                if b == 0:
                    wmap[l][1] = load_ffn(l)
                stage_b_combine(b, vots.pop(u), bmats.pop(u))
                if pending_proj is not None:
                    stage_final_proj(*pending_proj)
                    pending_proj = None
                if pending_stats is not None:
                    pending_proj = ((pending_stats,)
                                    + stage_final_stats(pending_stats))
                    pending_stats = None
                bcrow_n = (stage_b_chain(units[u + 1][1])
                           if u + 1 < len(units) else None)
                if u + 2 < len(units):
                    do_a(u + 2)
                if u + 1 < len(units):
                    bmats[u + 1] = stage_b_chain2(units[u + 1][1], bcrow_n)
                stage_b_back(l, b, wmap[l][1])
                if l == nlayers - 1:
                    pending_stats = b
            stage_final_proj(*pending_proj)
            stage_final_proj(pending_stats, *stage_final_stats(pending_stats))

    nc.compile()
    return nc


def _get_program(nbatch=4, nlayers=NL_TOT):
    key = (nbatch, nlayers)
    if key not in _BUILD_CACHE:
        _BUILD_CACHE[key] = _build(nbatch, nlayers)
    return _BUILD_CACHE[key]


def _prep_shared(inputs, nlayers):
    """Host-side input marshalling shared by all cores (weight layout/cast)."""
    f16 = np.float16
    wqkvo = np.stack([np.stack([np.asarray(inputs[n][l]).reshape(4, 128, 512)
                                for n in ("wq", "wk", "wv", "wo")])
                      for l in range(nlayers)]).astype(f16)
    w1 = np.stack([np.asarray(inputs["w1"][l]).reshape(4, 128, DFF)
                   for l in range(nlayers)]).astype(f16)
    w2 = np.stack([np.asarray(inputs["w2"][l]).reshape(16, 128, 512)
                   for l in range(nlayers)]).astype(f16)
    ew = np.asarray(inputs["embed_w"]).astype(f16)
    biases = np.zeros((128, nlayers * 16), np.float32)
    for l in range(nlayers):
        for w, n in enumerate(("bq", "bk", "bv", "bo")):
            arr = np.asarray(inputs[n][l])
            for m in range(4):
                biases[:, l * 16 + w * 4 + m] = arr[m * 128:(m + 1) * 128]
    pw_full = (np.asarray(inputs["ln_g"])[:, None]
               * np.asarray(inputs["proj_w"])).astype(np.float32)
    projw = pw_full.reshape(4, 128, 64).astype(f16)
    pbt = np.tile(np.asarray(inputs["proj_b"])[None, :], (128, 1)).astype(np.float32)
    # negated column sums of the (f16-rounded) ln_g-scaled projection, for
    # the factored final-LN: P = inv*A - (inv*mu)*colsum(pw)
    spwn = np.tile(-pw_full.astype(f16).astype(np.float32).sum(0)[None, :],
                   (128, 1)).astype(np.float32)
    onesk = np.full((128, 1), 1.0 / 1024, np.float32)
    # padded-cumsum edge ramps: front cs[i] = (i-12)*x0, back = total+k*xlast
    rampf = np.tile(np.arange(-PAD, 1, dtype=np.float32)[None, :], (128, 1))
    rampb = np.tile(np.arange(1, PAD + 1, dtype=np.float32)[None, :], (128, 1))
    ones512 = np.full((128, 1), 1.0 / 512, f16)
    # mod-1024 iota for the circulant shift blocks:
    # UG[u, 128g+v] = (128g + u - v) mod 1024 (integers <= 1023, f16-exact)
    u = np.arange(128)[:, None]
    v = np.arange(128)[None, :]
    ug = np.concatenate([(128 * g + u - v) % 1024 for g in range(8)],
                        axis=1).astype(f16)
    onesr = np.ones((1, 128), np.float32)
    return dict(wqkvo=wqkvo, w1=w1, w2=w2, ew=ew, biases=biases, projw=projw,
                pbt=pbt, rampf=rampf, rampb=rampb, ones512=ones512,
                ug=ug, onesr=onesr, spwn=spwn, onesk=onesk)


def _prep_x(xb):
    """(nb, L, CIN) fp32 -> (nb, 64, L+2) fp16 feature-major, circular padded."""
    xt = np.transpose(np.asarray(xb), (0, 2, 1))  # (nb, C, L)
    xe = np.concatenate([xt[:, :, -1:], xt, xt[:, :, :1]], axis=2)
    return xe.astype(np.float16)


def kernel(**inputs):
    from concourse.bass_utils import run_bass_kernel_spmd
    x = np.asarray(inputs["x"])
    B = x.shape[0]
    nbatch = B // NCORES
    nc = _get_program(nbatch, NL_TOT)
    shared = _prep_shared(inputs, NL_TOT)
    in_maps = []
    for c in range(NCORES):
        m = dict(shared)
        m["x"] = _prep_x(x[c * nbatch:(c + 1) * nbatch])
        in_maps.append(m)
    res = run_bass_kernel_spmd(nc, in_maps, core_ids=list(range(NCORES)))
    out = np.concatenate([res.results[c]["out"] for c in range(NCORES)], axis=0)
    return out.astype(np.float32)



# revision 111
# speedup vs baseline: 1.0490x; 1.0308x over previous
"""Autoformer encoder (nn_Autoformer_11441792876586) on 8 TRN2 NeuronCores.

Strategy: data-parallel over batch (4 batches/core). Per core, everything runs
in feature-major layout (channels on partitions, time on free dim):
  - all GEMMs: W stationary (din,dout)-tiles, activations moving -> fp16 in,
    fp32 PSUM accumulate
  - AutoCorrelation mean_corr via Gram matrix M' = K^T Q (c-contraction),
    block-diagonal sums C'_j accumulated straight from PSUM, DRAM shear read
    with ASCENDING element stride, ones-vector matmul partition reduction;
    top-8 via DVE Max8/MaxIndex; softmax on ACT/DVE
  - the weighted time-rolls sum_k tc_k*roll(V@Wo, d_k) run on the PE as a
    block-circulant matmul: V@Wo is produced TIME-major (voT, stationary =
    v16 time-slices), the 8 distinct 128x128 circulant blocks B_g[u,v] =
    sum_k tc_k*[(128g+u-v) mod 1024 == d_k] are built with one DVE
    is_equal+mult tensor_scalar per k against a host iota constant UG, and
    out[c, 128T+v] = sum_g voT[(T+g)%8]^T B_g accumulates in PSUM. This
    keeps gpsimd (Q7 ~27us/call dispatch stalls) entirely out of the kernel.
    bo is dropped: decomp(h + a + const) == decomp(h + a) since the
    edge-replicated moving average maps time-constants to themselves and
    softmax(tc) sums to 1.
  - all partition broadcasts via rank-1 ones-row matmuls (no gpsimd)
  - series_decomp via DVE prefix-scan cumsum + edge-replication corrections
  - residual stream h kept in f32 (the top-k selection has near-ties at
    ~1e-4 relative; an f16 stream drifts enough to flip them), cast to f16
    tiles right before each GEMM phase on the scalar engine
  - final my_Layernorm factored so the projection GEMMs depend only on the
    h cast, never a z-normalized tensor: out = P - mean_t(P) + pb with
    P = inv*(h^T pw) - (inv*mu)*colsum(pw); the inv/mu rows reach
    time-major per-partition form via a small DRAM round trip
Scheduling: all (layer, batch) units are software-pipelined 2 deep with
engines' in-order streams in mind. Per unit the emission order is
[combine(u)] [final-proj tail] [chain1(u+1): shear-read + s_j + top-8 +
softmax] [stage_a(u+2): QKV/Gram/voT + weight DMAs on the gpsimd queue]
[chain2(u+1): broadcast + B-build] [back(u): decomp/FFN/decomp], so the
PE always has the next unit's GEMMs queued while a unit's correlation
chain (DMA round trip + DVE small ops) resolves, and the B-build lands
in the DVE queue a full unit before its combine. gpsimd is used ONLY to
issue weight-load DMA descriptors: every Q7 compute op measured 3-27us
of dispatch latency and repeatedly regressed the span.
"""
import sys
if "/opt/trn_rl_repo" not in sys.path:
    sys.path.insert(0, "/opt/trn_rl_repo")
import hashlib as _hashlib
import os as _os
import numpy as np

# The neuronxcc NEFF cache keys on the HLO module hash, which does NOT cover
# the embedded bass program — a stale cache silently serves NEFFs built from
# an older kernel revision. Pin the cache to a dir derived from this file's
# contents so any source change forces a recompile.
_os.environ["NEURON_COMPILE_CACHE_URL"] = (
    "/tmp/neuron_cache_"
    + _hashlib.md5(open(__file__, "rb").read()).hexdigest()[:16])

L, CIN, D, DFF = 1024, 64, 512, 2048
NL_TOT, KMA, TOPK, EPS = 3, 25, 6, 1e-5
NCORES = 8
PAD = (KMA - 1) // 2  # 12
NTC = 2               # 512-wide time chunks per 1024
_BUILD_CACHE = {}


def _build(nbatch, nlayers):
    import concourse.bass as bass
    from concourse.bass import _add_dep_helper
    import concourse.bacc as bacc
    import concourse.tile as tile
    import concourse.mybir as mybir
    from contextlib import ExitStack

    DT = mybir.dt
    AF = mybir.ActivationFunctionType
    ALU = mybir.AluOpType
    AX = mybir.AxisListType
    F16, F32 = DT.float16, DT.float32

    nc = bacc.Bacc("TRN2", target_bir_lowering=False, debug=False,
                   num_devices=NCORES)

    # ---------------- I/O ----------------
    x_e = nc.dram_tensor("x", [nbatch, 64, L + 2], F16, kind="ExternalInput")
    ug_e = nc.dram_tensor("ug", [128, 1024], F16, kind="ExternalInput")
    onesr_e = nc.dram_tensor("onesr", [1, 128], F32, kind="ExternalInput")
    wqkvo_e = nc.dram_tensor("wqkvo", [nlayers, 4, 4, 128, 512], F16, kind="ExternalInput")
    w1_e = nc.dram_tensor("w1", [nlayers, 4, 128, DFF], F16, kind="ExternalInput")
    w2_e = nc.dram_tensor("w2", [nlayers, 16, 128, 512], F16, kind="ExternalInput")
    ew_e = nc.dram_tensor("ew", [3, 64, 512], F16, kind="ExternalInput")
    bias_e = nc.dram_tensor("biases", [128, nlayers * 16], F32, kind="ExternalInput")
    projw_e = nc.dram_tensor("projw", [4, 128, 64], F16, kind="ExternalInput")
    pbt_e = nc.dram_tensor("pbt", [128, 64], F32, kind="ExternalInput")
    spwn_e = nc.dram_tensor("spwn", [128, 64], F32, kind="ExternalInput")
    onesk_e = nc.dram_tensor("onesk", [128, 1], F32, kind="ExternalInput")
    # final-LN row transpose scratch (parity per batch)
    dfin = nc.dram_tensor("dfin", [2, 2, 1024], F32)
    rampf_e = nc.dram_tensor("rampf", [128, PAD + 1], F32, kind="ExternalInput")
    rampb_e = nc.dram_tensor("rampb", [128, PAD], F32, kind="ExternalInput")
    ones_e = nc.dram_tensor("ones512", [128, 1], F16, kind="ExternalInput")
    out_e = nc.dram_tensor("out", [nbatch, L, 64], F16, kind="ExternalOutput")
    # internal DRAM shear buffers (alternating parity per batch)
    dsh = [nc.dram_tensor(f"dsh{p}", [8, 128, 512], F16) for p in range(2)]

    with tile.TileContext(nc) as tc, ExitStack() as ctx:
        pool = ctx.enter_context(tc.tile_pool(name="sb", bufs=1))
        wpool = ctx.enter_context(tc.tile_pool(name="wp", bufs=1))
        pq = ctx.enter_context(tc.tile_pool(name="pq", bufs=5, space="PSUM"))
        pf = ctx.enter_context(tc.tile_pool(name="pf", bufs=3, space="PSUM"))

        # ------------- persistent constants -------------
        # ug + ew first: ug feeds the PE warm-up immediately and ew feeds
        # the embed; the rest can land while those run
        ug_sb = pool.tile([128, 1024], F16, tag="ug")
        nc.sync.dma_start(ug_sb[:], ug_e.ap())
        # PE clock warm-up: ~5us of dummy matmuls overlapping the remaining
        # constant/input DMAs, so the embed and first QKV stream at 2.4GHz
        # instead of the 1.2GHz cold clock (PE ramps after ~4us sustained)
        warm = pool.tile([128, 512], F16, tag="cw16", name="warm")
        for wi in range(12):
            psw = pq.tile([128, 512], F32, tag="ps")
            nc.tensor.matmul(psw[:], ug_sb[:, 0:128], ug_sb[:, 0:512],
                             start=True, stop=True)
            if wi == 11:
                nc.scalar.activation(warm[:], psw[:], AF.Identity)
        # ew lives in the g16 slot: it is only read during the embed stage,
        # before the first FFN allocates g16
        ew_sb = pool.tile([64, 3 * 512], F16, tag="g16", name="ew_sb")
        nc.sync.dma_start(ew_sb[:].rearrange("p (j c) -> p j c", j=3),
                          ew_e.ap().rearrange("j p c -> p j c"))
        # batch 0's input ahead of the lower-priority constants: the first
        # embed matmul otherwise idles ~16us while x(0) queues behind them
        x0_sb = pool.tile([64, L + 2], F16, tag="cs", name="x_sb")
        nc.sync.dma_start(x0_sb[:], x_e.ap()[0])
        bias_sb = pool.tile([128, nlayers * 16], F32, tag="bias")
        nc.sync.dma_start(bias_sb[:], bias_e.ap())
        projw_sb = pool.tile([128, 4 * 64], F16, tag="projw")
        nc.sync.dma_start(projw_sb[:].rearrange("p (k c) -> p k c", k=4),
                          projw_e.ap().rearrange("k p c -> p k c"))
        pbt_sb = pool.tile([128, 64], F32, tag="pbt")
        nc.sync.dma_start(pbt_sb[:], pbt_e.ap())
        spwn_sb = pool.tile([128, 64], F32, tag="spwn")
        nc.sync.dma_start(spwn_sb[:], spwn_e.ap())
        onesk_sb = pool.tile([128, 1], F32, tag="onesk")
        nc.sync.dma_start(onesk_sb[:], onesk_e.ap())
        rampf_sb = pool.tile([128, PAD + 1], F32, tag="rampf")
        nc.sync.dma_start(rampf_sb[:], rampf_e.ap())
        rampb_sb = pool.tile([128, PAD], F32, tag="rampb")
        nc.sync.dma_start(rampb_sb[:], rampb_e.ap())
        ones_sb = pool.tile([128, 1], F16, tag="ones")
        nc.sync.dma_start(ones_sb[:], ones_e.ap())
        onesr_sb = pool.tile([1, 128], F32, tag="onesr")
        nc.sync.dma_start(onesr_sb[:], onesr_e.ap())

        zero_writes = {0: [], 1: []}
        shear_writes = [{}, {}]
        shear_reads = [{}, {}]
        fin_reads = {}

        # persistent per-batch h (f32, feature-major: c-tile m at cols [1024m))
        h_b = [pool.tile([128, 4 * L], F32, tag=f"h{b}", name=f"h{b}")
               for b in range(nbatch)]

        def cast_h16(b, tag):
            """f16 snapshot of h for GEMM consumption, on the scalar engine
            (ACT Identity rounds f32->f16 the same as a DVE copy, and the
            scalar engine has headroom while DVE is the bottleneck)."""
            h16 = pool.tile([128, 4 * L], F16, tag=tag, name="h16")
            for m in range(4):
                nc.scalar.activation(h16[:, m * L:(m + 1) * L],
                                     h_b[b][:, m * L:(m + 1) * L], AF.Identity)
            return h16

        def gemm_512(dst_sb, dst_col, w_sb, w_base, rhs_sb, psum_pool,
                     bias_ap=None, act=None, nk=4):
            """dst[:, dst_col + m*L + tc*512] = act(sum_k W[k,m]^T @ rhs[k,tc]) + bias
            W blocks at w_sb[:, w_base + 512k + 128m]; rhs c-tile k at rhs_sb cols
            [L*k], time chunk tc at [512tc]. dst layout: c-tile m at [L*m]."""
            for m in range(4):
                for t in range(NTC):
                    ps = psum_pool.tile([128, 512], F32, tag="ps")
                    for k in range(nk):
                        nc.tensor.matmul(
                            ps[:],
                            w_sb[:, w_base + 512 * k + 128 * m:
                                 w_base + 512 * k + 128 * m + 128],
                            rhs_sb[:, L * k + 512 * t: L * k + 512 * t + 512],
                            start=(k == 0), stop=(k == nk - 1))
                    col = dst_col + L * m + 512 * t
                    nc.scalar.activation(dst_sb[:, col:col + 512], ps[:],
                                         act or AF.Identity,
                                         bias=bias_ap[m] if bias_ap else 0.0)

        def bias_aps(l, w):
            return [bias_sb[:, l * 16 + w * 4 + m: l * 16 + w * 4 + m + 1]
                    for m in range(4)]

        def decomp(b):
            """h <- h - moving_average(h) with edge replication; h=(128,4L) f32.

            The cs tile holds the EXACT padded cumsum (cs[i] = sum of the
            first i entries of the edge-replicated sequence, up to a global
            constant that cancels in the windowed diff): front pad =
            (i-12)*x0 via one tensor_scalar on an iota constant, scan with
            zero initial, back pad = total + k*x_last via one STT. No
            separate edge corrections or saved edge columns needed.
            DVE, not gpsimd: decomp is on the critical path and the Q7
            handoff adds ~2-4us latency per call."""
            h = h_b[b]
            for m in range(4):
                hx = h[:, m * L:(m + 1) * L]
                cs = pool.tile([128, L + 2 * PAD + 4], F32, tag="cs",
                               name="cs")
                nc.vector.tensor_scalar(cs[:, 0:PAD + 1], rampf_sb[:],
                                        hx[:, 0:1], None, ALU.mult)
                nc.vector.tensor_tensor_scan(cs[:, PAD + 1:PAD + 1 + L], hx, hx,
                                             0.0, ALU.add, ALU.bypass)
                nc.vector.scalar_tensor_tensor(
                    cs[:, PAD + 1 + L:PAD + 1 + L + PAD], rampb_sb[:],
                    hx[:, L - 1:L],
                    cs[:, PAD + L:PAD + 1 + L].to_broadcast((128, PAD)),
                    ALU.mult, ALU.add)
                # windowed sum A[t] = cs[t+25] - cs[t]; h = hx - A/25
                tmp = pool.tile([128, L], F32, tag="tmp", name="tmp")
                nc.vector.tensor_tensor(tmp[:], cs[:, KMA:KMA + L], cs[:, 0:L],
                                        ALU.subtract)
                nc.vector.scalar_tensor_tensor(hx, tmp[:], -1.0 / KMA, hx,
                                               ALU.mult, ALU.add)

        # ================= embed =================
        for b in range(nbatch):
            if b == 0:
                x_sb = x0_sb
            else:
                x_sb = pool.tile([64, L + 2], F16, tag="cs", name="x_sb")
                nc.sync.dma_start(x_sb[:], x_e.ap()[b])
            for m in range(4):
                for t in range(NTC):
                    ps = pq.tile([128, 512], F32, tag="ps")
                    for j in range(3):
                        nc.tensor.matmul(
                            ps[:],
                            ew_sb[0:64, 512 * j + 128 * m: 512 * j + 128 * m + 128],
                            x_sb[0:64, j + 512 * t: j + 512 * t + 512],
                            start=(j == 0), stop=(j == 2))
                    nc.scalar.activation(
                        h_b[b][:, L * m + 512 * t: L * m + 512 * t + 512],
                        ps[:], AF.Identity)

        # zero the shear scratch in DRAM once; emitted AFTER the embed so
        # these 2MB of writes queue behind the x loads the head needs first
        # (they only have to precede the first shear read, ~100us in)
        zero_sb = pool.tile([128, 512], F16, tag="cs", name="zero_sb")
        nc.vector.memset(zero_sb[:], 0.0)
        for p in range(2):
            for j in range(8):
                zero_writes[p].append(
                    nc.sync.dma_start(dsh[p].ap()[j], zero_sb[:]))

        # ================= layers (flat (l, b) unit pipeline) =================
        def load_qkvo(l):
            """Weight loads ride the GpSimd DMA queue (not Sync), so a load
            whose WAR-wait on the previous layer's last reads hasn't cleared
            can't head-of-line-block the Sync queue's shear DMAs. The qkvo
            load is emitted two units ahead (WAR on voT(l-1, last) clears
            early); the ffn load is emitted at unit (l, 0) itself since its
            WAR only clears at FFN2(l-1, last)."""
            qkvo_sb = wpool.tile([128, 4 * 2048], F16, tag="qkvo")
            nc.gpsimd.dma_start(
                qkvo_sb[:].rearrange("p (w k c) -> p w k c", w=4, k=4),
                wqkvo_e.ap()[l].rearrange("w k p c -> p w k c"))
            return qkvo_sb

        def load_ffn(l):
            ffn_sb = wpool.tile([128, 16384], F16, tag="ffn")
            nc.gpsimd.dma_start(
                ffn_sb[:, 0:8192].rearrange("p (k c) -> p k c", k=4),
                w1_e.ap()[l].rearrange("k p c -> p k c"))
            nc.gpsimd.dma_start(
                ffn_sb[:, 8192:16384].rearrange("p (k c) -> p k c", k=16),
                w2_e.ap()[l].rearrange("k p c -> p k c"))
            return ffn_sb

        if True:
            def stage_a(l, b, qkvo_sb):
                """PE-heavy front half: QKV gemms, Gram + shear write, voT.
                Emitted one batch AHEAD of stage_b(b-1) so the PE stream has
                queued work while b-1's correlation chain (DMA/DVE/ACT small
                ops) resolves."""
                h16 = cast_h16(b, "h16q")
                q16 = pool.tile([128, 4 * L], F16, tag="q16")
                k16 = pool.tile([128, 4 * L], F16, tag="k16")
                v16 = pool.tile([128, 4 * L], F16, tag="v16", name="v16")
                gemm_512(q16, 0, qkvo_sb, 0, h16, pq, bias_aps(l, 0))
                gemm_512(k16, 0, qkvo_sb, 2048, h16, pq, bias_aps(l, 1))
                gemm_512(v16, 0, qkvo_sb, 4096, h16, pq, bias_aps(l, 2))

                # ---- Gram M'_i = K_i^T Q (fp32 psum chunks, rounded to f16
                # per chunk); C'_j then summed from the f16 chunks on DVE:
                # C'_j[p,u] = sum_i M'_i[p, 128*((i+j)%8)+u]. A j-major
                # all-PSUM variant (32 128-wide matmuls per j) measured
                # +0.36ms: 128-wide moving exposes the 87ns LDWEIGHTS that a
                # 512-wide stream hides, so keep the 512-wide chunks.
                cacc = pool.tile([128, 1024], F32, tag="cacc")
                cw16 = pool.tile([128, 1024], F16, tag="cw16", name="cw16")
                for i in range(8):
                    for t2 in range(2):
                        psm = pq.tile([128, 512], F32, tag="ps")
                        for kc in range(4):
                            nc.tensor.matmul(
                                psm[:],
                                k16[:, L * kc + 128 * i: L * kc + 128 * i + 128],
                                q16[:, L * kc + 512 * t2: L * kc + 512 * t2 + 512],
                                start=(kc == 0), stop=(kc == 3))
                        chunk16 = pool.tile([128, 512], F16, tag="chunk16",
                                            bufs=2, name="chunk16")
                        nc.scalar.activation(chunk16[:], psm[:], AF.Identity)
                        for qb in range(4):
                            jq = 4 * t2 + qb          # q-time block in this chunk
                            j = (jq - i) % 8          # C'_j slice it feeds
                            blk = chunk16[:, 128 * qb:128 * qb + 128]
                            dst32 = cacc[:, 128 * j:128 * j + 128]
                            if i == 0:
                                nc.vector.tensor_copy(dst32, blk)
                            elif i < 7:
                                nc.vector.tensor_tensor(dst32, dst32, blk,
                                                        ALU.add)
                            else:
                                nc.vector.tensor_tensor(
                                    cw16[:, 128 * j:128 * j + 128], dst32, blk,
                                    ALU.add)
                # all 8 C'_j blocks in ONE 3-dim DMA (dst order (p, j, c))
                shw = bass.AP(tensor=dsh[b % 2].ap().tensor, offset=256,
                              ap=[[512, 128], [128 * 512, 8], [1, 128]])
                wr = nc.sync.dma_start(
                    shw, cw16[:].rearrange("p (j c) -> p j c", j=8))
                for j in range(8):
                    for prd in shear_reads[b % 2].get(j, []):
                        _add_dep_helper(wr.ins, prd.ins, sync=True,
                                        reason="shear WAR")
                    shear_writes[b % 2][j] = wr

                # voT = (v @ wo) TIME-major: voT_T[u, c] = sum_cin
                # v16[cin, 128T+u] * wo[cin, c]; stationary = v16 time-slices,
                # moving = wo [cin, 512 cout] (contiguous in qkvo_sb). bo is
                # dropped (decomp kills time-constant shifts, softmax sums
                # to 1). Emitted after the Gram so corr progress is preferred.
                vot = pool.tile([128, 8 * 512], F16, tag="vo2", bufs=2,
                                name="vot")
                for tb in range(8):
                    psv = pq.tile([128, 512], F32, tag="ps")
                    for kc in range(4):
                        nc.tensor.matmul(
                            psv[:],
                            v16[:, L * kc + 128 * tb: L * kc + 128 * tb + 128],
                            qkvo_sb[:, 6144 + 512 * kc: 6144 + 512 * kc + 512],
                            start=(kc == 0), stop=(kc == 3))
                    nc.scalar.activation(vot[:, 512 * tb:512 * tb + 512],
                                         psv[:], AF.Identity)
                return vot

            def stage_b_chain(b):
                # ---- shear read T[p, (j,u')] = D[j, p, p + 128 + u'] ----
                # ascending inner stride: 256 contiguous f16 = 512B/partition
                t16 = pool.tile([128, 8 * 256], F16, tag="t16", name="t16")
                shear_in = bass.AP(
                    tensor=dsh[b % 2].ap().tensor,
                    offset=128,
                    ap=[[513, 128], [128 * 512, 8], [1, 256]])
                rd = nc.sync.dma_start(
                    t16[:].rearrange("p (j t) -> p j t", j=8), shear_in)
                for j in range(8):
                    _add_dep_helper(rd.ins, shear_writes[b % 2][j].ins,
                                    sync=True, reason="shear RAW")
                    shear_reads[b % 2].setdefault(j, []).append(rd)
                for zw in zero_writes[b % 2]:
                    _add_dep_helper(rd.ins, zw.ins, sync=True,
                                    reason="shear zero RAW")

                # ---- s_j = ones(1/512)^T @ T_j  -> SBUF row (f32: the top-k
                # selection is precision-sensitive near ties) ----
                s_sb = pool.tile([1, 8 * 256], F32, tag="s_sb")
                for j2 in range(4):   # two j-blocks per 512-wide matmul
                    pss = pq.tile([1, 512], F32, tag="ps")
                    nc.tensor.matmul(pss[:], ones_sb[:],
                                     t16[:, 512 * j2:512 * j2 + 512],
                                     start=True, stop=True)
                    nc.scalar.activation(s_sb[:, 512 * j2:512 * j2 + 512],
                                         pss[:], AF.Identity)

                # ---- assemble r[128j+d] = sv_j[128+d] + sv_{j+1}[d] ----
                # (r32 shares the bmat slot: it is dead at max_index, before
                # this unit's bmat build; bmat(b) is dead once combine(b)'s
                # PE reads drain, well before r32(b+1) is written)
                r32 = pool.tile([1, L], F32, tag="bmat", name="r32")
                src_a = bass.AP(tensor=s_sb[:].tensor, offset=s_sb[:].offset + 128,
                                ap=[[s_sb[:].ap[0][0], 1], [256, 8], [1, 128]])
                nc.vector.tensor_copy(
                    r32[:].rearrange("p (j t) -> p j t", j=8), src_a)
                # += sv_{j+1}[d] for d>=1 ; j=0..6
                dst_b = bass.AP(tensor=r32[:].tensor, offset=r32[:].offset + 1,
                                ap=[[r32[:].ap[0][0], 1], [128, 7], [1, 127]])
                src_b = bass.AP(tensor=s_sb[:].tensor, offset=s_sb[:].offset + 257,
                                ap=[[s_sb[:].ap[0][0], 1], [256, 7], [1, 127]])
                nc.vector.tensor_tensor(dst_b, dst_b, src_b, ALU.add)
                # j=7 wraps to sv_0
                nc.vector.tensor_tensor(r32[:, 897:1024], r32[:, 897:1024],
                                        s_sb[:, 1:128], ALU.add)

                # ---- top-8 + softmax over top-6 ----
                top8 = pool.tile([1, 8], F32, tag="top8")
                idx8 = pool.tile([1, 8], DT.uint32, tag="idx8")
                nc.vector.max(top8[:], r32[:])
                nc.vector.max_index(idx8[:], top8[:], r32[:])
                negmax = pool.tile([1, 1], F32, tag="negmax")
                nc.vector.tensor_scalar_mul(negmax[:], top8[:, 0:1], -1.0)
                e6 = pool.tile([1, 8], F32, tag="e6")
                nc.scalar.activation(e6[:, 0:TOPK], top8[:, 0:TOPK], AF.Exp,
                                     bias=negmax[:], scale=1.0)
                se = pool.tile([1, 1], F32, tag="se")
                nc.vector.tensor_reduce(se[:], e6[:, 0:TOPK], AX.X, ALU.add)
                rse = pool.tile([1, 1], F32, tag="rse")
                nc.vector.reciprocal(rse[:], se[:])
                tc6 = pool.tile([1, 8], F32, tag="tc6")
                nc.vector.tensor_scalar_mul(tc6[:, 0:TOPK], e6[:, 0:TOPK], rse[:])
                # ---- broadcast tc + delays to all partitions via a rank-1
                # ones-row matmul (psum[p, j] = 1 * bcrow[0, j]) ----
                bcrow = pool.tile([1, 16], F32, tag="tcb", bufs=2)
                nc.vector.tensor_copy(bcrow[:, 0:TOPK], tc6[:, 0:TOPK])
                nc.vector.tensor_copy(bcrow[:, 8:8 + TOPK],
                                      idx8[0:1, 0:TOPK].bitcast(DT.int32))
                return bcrow

            def stage_b_chain2(b, bcrow):
                psbc = pq.tile([128, 16], F32, tag="ps")
                nc.tensor.matmul(psbc[:], onesr_sb[:], bcrow[:],
                                 start=True, stop=True)
                bc32 = pool.tile([128, 16], F32, tag="db8", bufs=2)
                nc.scalar.activation(bc32[:], psbc[:], AF.Identity)

                # ---- block-circulant shift matrix, 8 blocks of 128x128:
                # B[u, 128g+v] = sum_k tc_k * [(128g+u-v) mod 1024 == d_k]
                # (UG holds the mod-1024 iota; integers exact in f16) ----
                bmat = pool.tile([128, 1024], F16, tag="bmat", name="bmat")
                bsc = pool.tile([128, 1024], F16, tag="bsc", name="bsc")
                for k in range(TOPK):
                    dst = bmat if k == 0 else bsc
                    nc.vector.tensor_scalar(dst[:], ug_sb[:],
                                            bc32[:, 8 + k:9 + k],
                                            bc32[:, k:k + 1],
                                            ALU.is_equal, ALU.mult)
                    if k:
                        nc.vector.tensor_tensor(bmat[:], bmat[:], bsc[:],
                                                ALU.add)
                return bmat

            def stage_b_combine(b, vot, bmat):
                # ---- combine: h[c, 128T+v] += sum_g voT[(T+g)%8][u, c]^T
                # @ B_g[u, v], accumulated over g in PSUM ----
                for m in range(4):
                    for tb4 in range(2):
                        # four T-blocks share one psum bank (four accumulation
                        # groups in disjoint column quarters) so the h-update
                        # is one 512-wide DVE op instead of four 128-wide
                        psc = pq.tile([128, 512], F32, tag="ps")
                        for qtr in range(4):
                            tb = 4 * tb4 + qtr
                            for g in range(8):
                                j = (tb + g) % 8
                                nc.tensor.matmul(
                                    psc[:, 128 * qtr:128 * qtr + 128],
                                    vot[:, 512 * j + 128 * m:
                                        512 * j + 128 * m + 128],
                                    bmat[:, 128 * g:128 * g + 128],
                                    start=(g == 0), stop=(g == 7))
                        col = L * m + 512 * tb4
                        nc.vector.tensor_tensor(h_b[b][:, col:col + 512],
                                                h_b[b][:, col:col + 512],
                                                psc[:], ALU.add)

            def stage_b_back(l, b, ffn_sb):
                decomp(b)

                # ================= FFN =================
                # h16f reuses the h16q slot: h16q(b+1), cast in the pipelined
                # stage_a(b+1) emitted just before this, is dead once b+1's
                # QKV matmuls (earlier in the PE stream) have read it
                h16f = cast_h16(b, "h16q")
                for t in range(NTC):
                    # FFN intermediate lives in the dead q16/k16 slots (their
                    # tiles are consumed by stage_a's Gram, which for batch
                    # b+1 runs earlier in the PE stream than this FFN)
                    g16a = pool.tile([128, 8 * 512], F16, tag="q16",
                                     name="g16a")
                    g16b = pool.tile([128, 8 * 512], F16, tag="k16",
                                     name="g16b")
                    ghalf = [g16a, g16b]
                    for dm in range(16):
                        ps = pf.tile([128, 512], F32, tag="psf")
                        for k in range(4):
                            nc.tensor.matmul(
                                ps[:],
                                ffn_sb[:, 2048 * k + 128 * dm:
                                       2048 * k + 128 * dm + 128],
                                h16f[:, L * k + 512 * t: L * k + 512 * t + 512],
                                start=(k == 0), stop=(k == 3))
                        nc.scalar.activation(
                            ghalf[dm // 8][:, 512 * (dm % 8):
                                           512 * (dm % 8) + 512],
                            ps[:], AF.Gelu)
                    for m in range(4):
                        psy = pf.tile([128, 512], F32, tag="psf")
                        for k in range(16):
                            nc.tensor.matmul(
                                psy[:],
                                ffn_sb[:, 8192 + 512 * k + 128 * m:
                                       8192 + 512 * k + 128 * m + 128],
                                ghalf[k // 8][:, 512 * (k % 8):
                                              512 * (k % 8) + 512],
                                start=(k == 0), stop=(k == 15))
                        col = L * m + 512 * t
                        nc.vector.tensor_tensor(h_b[b][:, col:col + 512],
                                                h_b[b][:, col:col + 512],
                                                psy[:], ALU.add)
                decomp(b)

            def stage_final_stats(b):
                # ======== final my_Layernorm + projection for batch b ======
                hf = cast_h16(b, "h16q")
                # mu, sumsq rows via ones-matmuls (scaled by 1/512). stat
                # shares the cacc slot and mu2/sd the r32 slot: both tags'
                # pipelined uses (A-unit Gram / front r-assembly) interleave
                # in emission order with no cross-engine stalls.
                stat = pool.tile([1, 2 * L], F32, tag="cacc", name="stat")
                # square on DVE (one f16 2x-mode op), not ACT: the ssq stats
                # matmuls were stalling ~2.5us each behind gelu+cast traffic
                # in the ACT queue waiting for Square
                h2 = pool.tile([128, 4 * L], F16, tag="q16", name="h2")
                nc.vector.tensor_tensor(h2[:], hf[:], hf[:], ALU.mult)
                for which, src in ((0, hf), (1, h2)):
                    for t in range(NTC):
                        ps = pq.tile([1, 512], F32, tag="ps")
                        for k in range(4):
                            nc.tensor.matmul(
                                ps[:], ones_sb[:],
                                src[:, L * k + 512 * t: L * k + 512 * t + 512],
                                start=(k == 0), stop=(k == 3))
                        nc.vector.tensor_copy(
                            stat[:, which * L + 512 * t:
                                 which * L + 512 * t + 512], ps[:])
                # The projection is factored so its GEMMs depend only on hf,
                # never on a z-normalized tensor:
                #   out = P - mean_t(P) + pb,
                #   P[t,co] = inv[t]*A[t,co] - (inv*mu)[t]*colsum(pw)[co],
                #   A = hf^T @ pw.
                # The raw mu/ssq rows go time-major via a DRAM round trip;
                # the var/sqrt/recip/w math happens AFTER the transpose on
                # [128, 8] tiles (128 lanes) instead of 1-lane [1, 1024] rows.
                wr = nc.sync.dma_start(
                    dfin.ap()[b % 2].rearrange("r t -> (r t)"), stat[:])
                for prd in fin_reads.get(b % 2, []):
                    _add_dep_helper(wr.ins, prd.ins, sync=True,
                                    reason="fin WAR")
                # A[t, co] blocks: stationary = hf time-slices (ready at the
                # cast -- no wait on any normalization)
                a32 = pool.tile([128, 8 * 64], F32, tag="g16", name="a32")
                for tt in range(8):
                    ps = pf.tile([128, 512], F32, tag="psf")
                    for k in range(4):
                        nc.tensor.matmul(
                            ps[:, 0:64],
                            hf[:, L * k + 128 * tt: L * k + 128 * tt + 128],
                            projw_sb[:, 64 * k:64 * k + 64],
                            start=(k == 0), stop=(k == 3))
                    nc.scalar.activation(a32[:, 64 * tt:64 * tt + 64],
                                         ps[:, 0:64], AF.Identity)
                return a32, wr

            def stage_final_proj(b, a32, wr):
                # P assembly + time-mean subtraction + output store; emitted
                # one unit later so the DRAM round trip has fully landed
                invw = pool.tile([128, 16], F32, tag="db8", bufs=2,
                                 name="invw")
                for r in range(2):   # cols 0:8 = mu_t, 8:16 = ssq_t
                    rdap = bass.AP(tensor=dfin.ap().tensor,
                                   offset=2048 * (b % 2) + 1024 * r,
                                   ap=[[1, 128], [128, 8]])
                    rd = nc.sync.dma_start(invw[:, 8 * r:8 * r + 8], rdap)
                    _add_dep_helper(rd.ins, wr.ins, sync=True,
                                    reason="fin RAW")
                    fin_reads.setdefault(b % 2, []).append(rd)
                # inv = 1/sqrt(ssq - mu^2 + eps); w = inv*mu  (all [128, 8])
                iwt = pool.tile([128, 16], F32, tag="mz", name="iwt")
                nc.vector.tensor_tensor(iwt[:, 0:8], invw[:, 0:8],
                                        invw[:, 0:8], ALU.mult)
                nc.vector.tensor_tensor(iwt[:, 0:8], invw[:, 8:16],
                                        iwt[:, 0:8], ALU.subtract)
                nc.vector.tensor_scalar_add(iwt[:, 0:8], iwt[:, 0:8], EPS)
                nc.scalar.activation(iwt[:, 0:8], iwt[:, 0:8], AF.Sqrt)
                nc.vector.reciprocal(iwt[:, 0:8], iwt[:, 0:8])
                nc.vector.tensor_tensor(iwt[:, 8:16], iwt[:, 0:8],
                                        invw[:, 0:8], ALU.mult)
                for tt in range(8):
                    blk = a32[:, 64 * tt:64 * tt + 64]
                    nc.vector.tensor_scalar(blk, blk, iwt[:, tt:tt + 1],
                                            None, ALU.mult)
                    nc.vector.scalar_tensor_tensor(
                        blk, spwn_sb[:], iwt[:, 8 + tt:9 + tt], blk,
                        ALU.mult, ALU.add)
                # mean_t(P) row via ones(1/1024) matmuls, then pb - mean
                psm = pq.tile([1, 64], F32, tag="ps", padded_shape=[1, 512])
                for tt in range(8):
                    nc.tensor.matmul(psm[:], onesk_sb[:],
                                     a32[:, 64 * tt:64 * tt + 64],
                                     start=(tt == 0), stop=(tt == 7))
                pbm = pool.tile([1, 64], F32, tag="tcb", bufs=2, name="pbm")
                nc.vector.tensor_tensor(pbm[:], pbt_sb[0:1, :], psm[:],
                                        ALU.subtract)
                psb = pq.tile([128, 64], F32, tag="ps")
                nc.tensor.matmul(psb[:], onesr_sb[:], pbm[:],
                                 start=True, stop=True)
                o16 = pool.tile([128, 8 * 64], F16, tag="o32", name="o16")
                for tt in range(8):
                    nc.vector.tensor_tensor(o16[:, 64 * tt:64 * tt + 64],
                                            a32[:, 64 * tt:64 * tt + 64],
                                            psb[:], ALU.add)
                # one 3-dim DMA: out[b, 128*tt + p, co] <- o16[p, 64*tt + co]
                odst = bass.AP(tensor=out_e.ap().tensor, offset=b * L * 64,
                               ap=[[64, 128], [8192, 8], [1, 64]])
                nc.sync.dma_start(odst,
                                  o16[:].rearrange("p (t c) -> p t c", t=8))

            # -- software pipeline over ALL (l, b) units, 2 units deep: the
            # PE stream per unit is [combine(u)][QKV/Gram/voT(u+2)][FFN(u)],
            # so u's correlation chain (DMA/DVE/ACT) hides under u+1/u+2 PE
            # work, u's decomp latency hides under u+2's QKV/Gram, and layer
            # boundaries don't drain the pipeline (weights for l+1 load via
            # the gpsimd DMA queue two units ahead). The final my_Layernorm +
            # projection for batch b rides the tail of the last layer's unit.
            units = [(l, b) for l in range(nlayers) for b in range(nbatch)]
            vots, wmap = {}, {}

            def do_a(u):
                l, b = units[u]
                if b == 0:
                    wmap[l] = [load_qkvo(l), None]
                vots[u] = stage_a(l, b, wmap[l][0])

            # chain(u+1) is emitted a full unit before combine(u+1) so its
            # small-op DVE tail (r-assembly, top-8, softmax, B-build) runs
            # ahead of unit u's decomp work in the DVE queue and the combine
            # matmuls never wait on it; the shear-read DMA also stays ahead
            # of unit u+2's shear-writes in the Sync queue.
            do_a(0)
            if len(units) > 1:
                do_a(1)
            # finals are double-deferred: stats(b) runs after combine(u+1)
            # (so its matmuls never wait on decomp2(u)-gated casts ahead of
            # the combine), proj(b) after combine(u+2)
            bmats = {0: stage_b_chain2(0, stage_b_chain(0))}
            pending_stats = None
            pending_proj = None
            for u, (l, b) in enumerate(units):
                if b == 0:
                    wmap[l][1] = load_ffn(l)
                stage_b_combine(b, vots.pop(u), bmats.pop(u))
                if pending_proj is not None:
                    stage_final_proj(*pending_proj)
                    pending_proj = None
                if pending_stats is not None:
                    pending_proj = ((pending_stats,)
                                    + stage_final_stats(pending_stats))
                    pending_stats = None
                bcrow_n = (stage_b_chain(units[u + 1][1])
                           if u + 1 < len(units) else None)
                if u + 2 < len(units):
                    do_a(u + 2)
                if u + 1 < len(units):
                    bmats[u + 1] = stage_b_chain2(units[u + 1][1], bcrow_n)
                stage_b_back(l, b, wmap[l][1])
                if l == nlayers - 1:
                    pending_stats = b
            stage_final_proj(*pending_proj)
            stage_final_proj(pending_stats, *stage_final_stats(pending_stats))

    nc.compile()
    return nc


def _get_program(nbatch=4, nlayers=NL_TOT):
    key = (nbatch, nlayers)
    if key not in _BUILD_CACHE:
        _BUILD_CACHE[key] = _build(nbatch, nlayers)
    return _BUILD_CACHE[key]


def _prep_shared(inputs, nlayers):
    """Host-side input marshalling shared by all cores (weight layout/cast)."""
    f16 = np.float16
    wqkvo = np.stack([np.stack([np.asarray(inputs[n][l]).reshape(4, 128, 512)
                                for n in ("wq", "wk", "wv", "wo")])
                      for l in range(nlayers)]).astype(f16)
    w1 = np.stack([np.asarray(inputs["w1"][l]).reshape(4, 128, DFF)
                   for l in range(nlayers)]).astype(f16)
    w2 = np.stack([np.asarray(inputs["w2"][l]).reshape(16, 128, 512)
                   for l in range(nlayers)]).astype(f16)
    ew = np.asarray(inputs["embed_w"]).astype(f16)
    biases = np.zeros((128, nlayers * 16), np.float32)
    for l in range(nlayers):
        for w, n in enumerate(("bq", "bk", "bv", "bo")):
            arr = np.asarray(inputs[n][l])
            for m in range(4):
                biases[:, l * 16 + w * 4 + m] = arr[m * 128:(m + 1) * 128]
    pw_full = (np.asarray(inputs["ln_g"])[:, None]
               * np.asarray(inputs["proj_w"])).astype(np.float32)
    projw = pw_full.reshape(4, 128, 64).astype(f16)
    pbt = np.tile(np.asarray(inputs["proj_b"])[None, :], (128, 1)).astype(np.float32)
    # negated column sums of the (f16-rounded) ln_g-scaled projection, for
    # the factored final-LN: P = inv*A - (inv*mu)*colsum(pw)
    spwn = np.tile(-pw_full.astype(f16).astype(np.float32).sum(0)[None, :],
                   (128, 1)).astype(np.float32)
    onesk = np.full((128, 1), 1.0 / 1024, np.float32)
    # padded-cumsum edge ramps: front cs[i] = (i-12)*x0, back = total+k*xlast
    rampf = np.tile(np.arange(-PAD, 1, dtype=np.float32)[None, :], (128, 1))
    rampb = np.tile(np.arange(1, PAD + 1, dtype=np.float32)[None, :], (128, 1))
    ones512 = np.full((128, 1), 1.0 / 512, f16)
    # mod-1024 iota for the circulant shift blocks:
    # UG[u, 128g+v] = (128g + u - v) mod 1024 (integers <= 1023, f16-exact)
    u = np.arange(128)[:, None]
    v = np.arange(128)[None, :]
    ug = np.concatenate([(128 * g + u - v) % 1024 for g in range(8)],
                        axis=1).astype(f16)
    onesr = np.ones((1, 128), np.float32)
    return dict(wqkvo=wqkvo, w1=w1, w2=w2, ew=ew, biases=biases, projw=projw,
                pbt=pbt, rampf=rampf, rampb=rampb, ones512=ones512,
                ug=ug, onesr=onesr, spwn=spwn, onesk=onesk)


def _prep_x(xb):
    """(nb, L, CIN) fp32 -> (nb, 64, L+2) fp16 feature-major, circular padded."""
    xt = np.transpose(np.asarray(xb), (0, 2, 1))  # (nb, C, L)
    xe = np.concatenate([xt[:, :, -1:], xt, xt[:, :, :1]], axis=2)
    return xe.astype(np.float16)


def kernel(**inputs):
    from concourse.bass_utils import run_bass_kernel_spmd
    x = np.asarray(inputs["x"])
    B = x.shape[0]
    nbatch = B // NCORES
    nc = _get_program(nbatch, NL_TOT)
    shared = _prep_shared(inputs, NL_TOT)
    in_maps = []
    for c in range(NCORES):
        m = dict(shared)
        m["x"] = _prep_x(x[c * nbatch:(c + 1) * nbatch])
        in_maps.append(m)
    res = run_bass_kernel_spmd(nc, in_maps, core_ids=list(range(NCORES)))
    out = np.concatenate([res.results[c]["out"] for c in range(NCORES)], axis=0)
    return out.astype(np.float32)



# revision 113
# speedup vs baseline: 1.1321x; 1.0793x over previous
"""Autoformer encoder (nn_Autoformer_11441792876586) on 8 TRN2 NeuronCores.

Strategy: data-parallel over batch (4 batches/core). Per core, everything runs
in feature-major layout (channels on partitions, time on free dim):
  - all GEMMs: W stationary (din,dout)-tiles, activations moving -> fp16 in,
    fp32 PSUM accumulate
  - AutoCorrelation mean_corr via Gram matrix M' = K^T Q (c-contraction),
    block-diagonal sums C'_j accumulated straight from PSUM, DRAM shear read
    with ASCENDING element stride, ones-vector matmul partition reduction;
    top-8 via DVE Max8/MaxIndex; softmax on ACT/DVE
  - the weighted time-rolls sum_k tc_k*roll(V@Wo, d_k) run on the PE as a
    block-circulant matmul: V@Wo is produced TIME-major (voT, stationary =
    v16 time-slices), the 8 distinct 128x128 circulant blocks B_g[u,v] =
    sum_k tc_k*[(128g+u-v) mod 1024 == d_k] are built with one DVE
    is_equal+mult tensor_scalar per k against a host iota constant UG, and
    out[c, 128T+v] = sum_g voT[(T+g)%8]^T B_g accumulates in PSUM. This
    keeps gpsimd (Q7 ~27us/call dispatch stalls) entirely out of the kernel.
    bo is dropped: decomp(h + a + const) == decomp(h + a) since the
    edge-replicated moving average maps time-constants to themselves and
    softmax(tc) sums to 1.
  - all partition broadcasts via rank-1 ones-row matmuls (no gpsimd)
  - series_decomp via DVE prefix-scan cumsum + edge-replication corrections
  - residual stream h kept in f32 (the top-k selection has near-ties at
    ~1e-4 relative; an f16 stream drifts enough to flip them), cast to f16
    tiles right before each GEMM phase on the scalar engine
  - final my_Layernorm factored so the projection GEMMs depend only on the
    h cast, never a z-normalized tensor: out = P - mean_t(P) + pb with
    P = inv*(h^T pw) - (inv*mu)*colsum(pw); the inv/mu rows reach
    time-major per-partition form via a small DRAM round trip
Scheduling: all (layer, batch) units are software-pipelined 2 deep with
engines' in-order streams in mind. Per unit the emission order is
[combine(u)] [final-proj tail] [chain1(u+1): shear-read + s_j + top-8 +
softmax] [stage_a(u+2): QKV/Gram/voT + weight DMAs on the gpsimd queue]
[chain2(u+1): broadcast + B-build] [back(u): decomp/FFN/decomp], so the
PE always has the next unit's GEMMs queued while a unit's correlation
chain (DMA round trip + DVE small ops) resolves, and the B-build lands
in the DVE queue a full unit before its combine. gpsimd is used ONLY to
issue weight-load DMA descriptors: every Q7 compute op measured 3-27us
of dispatch latency and repeatedly regressed the span.
"""
import sys
if "/opt/trn_rl_repo" not in sys.path:
    sys.path.insert(0, "/opt/trn_rl_repo")
import hashlib as _hashlib
import os as _os
import numpy as np

# The neuronxcc NEFF cache keys on the HLO module hash, which does NOT cover
# the embedded bass program — a stale cache silently serves NEFFs built from
# an older kernel revision. Pin the cache to a dir derived from this file's
# contents so any source change forces a recompile.
_os.environ["NEURON_COMPILE_CACHE_URL"] = (
    "/tmp/neuron_cache_"
    + _hashlib.md5(open(__file__, "rb").read()).hexdigest()[:16])

L, CIN, D, DFF = 1024, 64, 512, 2048
NL_TOT, KMA, TOPK, EPS = 3, 25, 6, 1e-5
NCORES = 8
PAD = (KMA - 1) // 2  # 12
NTC = 2               # 512-wide time chunks per 1024
_BUILD_CACHE = {}


def _build(nbatch, nlayers):
    import concourse.bass as bass
    from concourse.bass import _add_dep_helper
    import concourse.bacc as bacc
    import concourse.tile as tile
    import concourse.mybir as mybir
    from contextlib import ExitStack

    DT = mybir.dt
    AF = mybir.ActivationFunctionType
    ALU = mybir.AluOpType
    AX = mybir.AxisListType
    F16, F32 = DT.float16, DT.float32

    nc = bacc.Bacc("TRN2", target_bir_lowering=False, debug=False,
                   num_devices=NCORES)

    # ---------------- I/O ----------------
    x_e = nc.dram_tensor("x", [nbatch, 64, L + 2], F16, kind="ExternalInput")
    ug_e = nc.dram_tensor("ug", [128, 1024], F16, kind="ExternalInput")
    onesr_e = nc.dram_tensor("onesr", [1, 128], F32, kind="ExternalInput")
    wqkvo_e = nc.dram_tensor("wqkvo", [nlayers, 4, 4, 128, 512], F16, kind="ExternalInput")
    w1_e = nc.dram_tensor("w1", [nlayers, 4, 128, DFF], F16, kind="ExternalInput")
    w2_e = nc.dram_tensor("w2", [nlayers, 16, 128, 512], F16, kind="ExternalInput")
    ew_e = nc.dram_tensor("ew", [3, 64, 512], F16, kind="ExternalInput")
    bias_e = nc.dram_tensor("biases", [128, nlayers * 16], F32, kind="ExternalInput")
    projw_e = nc.dram_tensor("projw", [4, 128, 64], F16, kind="ExternalInput")
    pbt_e = nc.dram_tensor("pbt", [128, 64], F32, kind="ExternalInput")
    spwn_e = nc.dram_tensor("spwn", [128, 64], F32, kind="ExternalInput")
    onesk_e = nc.dram_tensor("onesk", [128, 1], F32, kind="ExternalInput")
    # final-LN row transpose scratch (parity per batch)
    dfin = nc.dram_tensor("dfin", [2, 2, 1024], F32)
    rampf_e = nc.dram_tensor("rampf", [128, PAD + 1], F32, kind="ExternalInput")
    rampb_e = nc.dram_tensor("rampb", [128, PAD], F32, kind="ExternalInput")
    ones_e = nc.dram_tensor("ones512", [128, 1], F16, kind="ExternalInput")
    out_e = nc.dram_tensor("out", [nbatch, L, 64], F16, kind="ExternalOutput")
    # internal DRAM shear buffers (alternating parity per batch)
    dsh = [nc.dram_tensor(f"dsh{p}", [8, 128, 512], F16) for p in range(2)]

    with tile.TileContext(nc) as tc, ExitStack() as ctx:
        pool = ctx.enter_context(tc.tile_pool(name="sb", bufs=1))
        wpool = ctx.enter_context(tc.tile_pool(name="wp", bufs=1))
        pq = ctx.enter_context(tc.tile_pool(name="pq", bufs=5, space="PSUM"))
        pf = ctx.enter_context(tc.tile_pool(name="pf", bufs=3, space="PSUM"))

        # ------------- persistent constants -------------
        # ug + ew first: ug feeds the PE warm-up immediately and ew feeds
        # the embed; the rest can land while those run
        ug_sb = pool.tile([128, 1024], F16, tag="ug")
        nc.sync.dma_start(ug_sb[:], ug_e.ap())
        # PE clock warm-up: ~5us of dummy matmuls overlapping the remaining
        # constant/input DMAs, so the embed and first QKV stream at 2.4GHz
        # instead of the 1.2GHz cold clock (PE ramps after ~4us sustained)
        warm = pool.tile([128, 512], F16, tag="cw16", name="warm")
        for wi in range(12):
            psw = pq.tile([128, 512], F32, tag="ps")
            nc.tensor.matmul(psw[:], ug_sb[:, 0:128], ug_sb[:, 0:512],
                             start=True, stop=True)
            if wi == 11:
                nc.scalar.activation(warm[:], psw[:], AF.Identity)
        # ew lives in the g16 slot: it is only read during the embed stage,
        # before the first FFN allocates g16
        ew_sb = pool.tile([64, 3 * 512], F16, tag="g16", name="ew_sb")
        nc.sync.dma_start(ew_sb[:].rearrange("p (j c) -> p j c", j=3),
                          ew_e.ap().rearrange("j p c -> p j c"))
        # batch 0's input ahead of the lower-priority constants: the first
        # embed matmul otherwise idles ~16us while x(0) queues behind them
        x0_sb = pool.tile([64, L + 2], F16, tag="cs", name="x_sb")
        nc.sync.dma_start(x0_sb[:], x_e.ap()[0])
        bias_sb = pool.tile([128, nlayers * 16], F32, tag="bias")
        nc.sync.dma_start(bias_sb[:], bias_e.ap())
        projw_sb = pool.tile([128, 4 * 64], F16, tag="projw")
        nc.sync.dma_start(projw_sb[:].rearrange("p (k c) -> p k c", k=4),
                          projw_e.ap().rearrange("k p c -> p k c"))
        pbt_sb = pool.tile([128, 64], F32, tag="pbt")
        nc.sync.dma_start(pbt_sb[:], pbt_e.ap())
        spwn_sb = pool.tile([128, 64], F32, tag="spwn")
        nc.sync.dma_start(spwn_sb[:], spwn_e.ap())
        onesk_sb = pool.tile([128, 1], F32, tag="onesk")
        nc.sync.dma_start(onesk_sb[:], onesk_e.ap())
        rampf_sb = pool.tile([128, PAD + 1], F32, tag="rampf")
        nc.sync.dma_start(rampf_sb[:], rampf_e.ap())
        rampb_sb = pool.tile([128, PAD], F32, tag="rampb")
        nc.sync.dma_start(rampb_sb[:], rampb_e.ap())
        ones_sb = pool.tile([128, 1], F16, tag="ones")
        nc.sync.dma_start(ones_sb[:], ones_e.ap())
        onesr_sb = pool.tile([1, 128], F32, tag="onesr")
        nc.sync.dma_start(onesr_sb[:], onesr_e.ap())

        zero_writes = {0: [], 1: []}
        shear_writes = [{}, {}]
        shear_reads = [{}, {}]
        fin_reads = {}

        # persistent per-batch h (f32, feature-major: c-tile m at cols [1024m))
        h_b = [pool.tile([128, 4 * L], F32, tag=f"h{b}", name=f"h{b}")
               for b in range(nbatch)]

        def cast_h16(b, tag):
            """f16 snapshot of h for GEMM consumption, on the scalar engine
            (ACT Identity rounds f32->f16 the same as a DVE copy, and the
            scalar engine has headroom while DVE is the bottleneck)."""
            h16 = pool.tile([128, 4 * L], F16, tag=tag, name="h16")
            for m in range(4):
                nc.scalar.activation(h16[:, m * L:(m + 1) * L],
                                     h_b[b][:, m * L:(m + 1) * L], AF.Identity)
            return h16

        def gemm_512(dst_sb, dst_col, w_sb, w_base, rhs_sb, psum_pool,
                     bias_ap=None, act=None, nk=4):
            """dst[:, dst_col + m*L + tc*512] = act(sum_k W[k,m]^T @ rhs[k,tc]) + bias
            W blocks at w_sb[:, w_base + 512k + 128m]; rhs c-tile k at rhs_sb cols
            [L*k], time chunk tc at [512tc]. dst layout: c-tile m at [L*m]."""
            for m in range(4):
                for t in range(NTC):
                    ps = psum_pool.tile([128, 512], F32, tag="ps")
                    for k in range(nk):
                        nc.tensor.matmul(
                            ps[:],
                            w_sb[:, w_base + 512 * k + 128 * m:
                                 w_base + 512 * k + 128 * m + 128],
                            rhs_sb[:, L * k + 512 * t: L * k + 512 * t + 512],
                            start=(k == 0), stop=(k == nk - 1))
                    col = dst_col + L * m + 512 * t
                    nc.scalar.activation(dst_sb[:, col:col + 512], ps[:],
                                         act or AF.Identity,
                                         bias=bias_ap[m] if bias_ap else 0.0)

        def bias_aps(l, w):
            return [bias_sb[:, l * 16 + w * 4 + m: l * 16 + w * 4 + m + 1]
                    for m in range(4)]

        def decomp(b):
            """h <- h - moving_average(h) with edge replication; h=(128,4L) f32.

            The cs tile holds the EXACT padded cumsum (cs[i] = sum of the
            first i entries of the edge-replicated sequence, up to a global
            constant that cancels in the windowed diff): front pad =
            (i-12)*x0 via one tensor_scalar on an iota constant, scan with
            zero initial, back pad = total + k*x_last via one STT. No
            separate edge corrections or saved edge columns needed.
            DVE, not gpsimd: decomp is on the critical path and the Q7
            handoff adds ~2-4us latency per call."""
            h = h_b[b]
            for m in range(4):
                hx = h[:, m * L:(m + 1) * L]
                cs = pool.tile([128, L + 2 * PAD + 4], F32, tag="cs",
                               name="cs")
                nc.vector.tensor_scalar(cs[:, 0:PAD + 1], rampf_sb[:],
                                        hx[:, 0:1], None, ALU.mult)
                nc.vector.tensor_tensor_scan(cs[:, PAD + 1:PAD + 1 + L], hx, hx,
                                             0.0, ALU.add, ALU.bypass)
                nc.vector.scalar_tensor_tensor(
                    cs[:, PAD + 1 + L:PAD + 1 + L + PAD], rampb_sb[:],
                    hx[:, L - 1:L],
                    cs[:, PAD + L:PAD + 1 + L].to_broadcast((128, PAD)),
                    ALU.mult, ALU.add)
                # windowed sum A[t] = cs[t+25] - cs[t]; h = hx - A/25
                tmp = pool.tile([128, L], F32, tag="tmp", name="tmp")
                nc.vector.tensor_tensor(tmp[:], cs[:, KMA:KMA + L], cs[:, 0:L],
                                        ALU.subtract)
                nc.vector.scalar_tensor_tensor(hx, tmp[:], -1.0 / KMA, hx,
                                               ALU.mult, ALU.add)

        # ================= embed =================
        # x tiles alternate between the cs and bmat slots (bmat's first real
        # user is chain(0,0)'s r32, far later): batch b+1's x DMA then has no
        # WAR on batch b's embed reads, so the DMAs prefetch and the PE
        # stays hot through the embed instead of re-gating to 1.2GHz in the
        # ~10us inter-batch stalls the single-slot rotation caused
        for b in range(nbatch):
            if b == 0:
                x_sb = x0_sb
            else:
                x_sb = pool.tile([64, L + 2], F16,
                                 tag=("cs", "bmat")[b % 2], name="x_sb")
                nc.sync.dma_start(x_sb[:], x_e.ap()[b])
            for m in range(4):
                for t in range(NTC):
                    ps = pq.tile([128, 512], F32, tag="ps")
                    for j in range(3):
                        nc.tensor.matmul(
                            ps[:],
                            ew_sb[0:64, 512 * j + 128 * m: 512 * j + 128 * m + 128],
                            x_sb[0:64, j + 512 * t: j + 512 * t + 512],
                            start=(j == 0), stop=(j == 2))
                    nc.scalar.activation(
                        h_b[b][:, L * m + 512 * t: L * m + 512 * t + 512],
                        ps[:], AF.Identity)

        # zero the shear scratch in DRAM once; emitted AFTER the embed so
        # these 2MB of writes queue behind the x loads the head needs first
        # (they only have to precede the first shear read, ~100us in)
        zero_sb = pool.tile([128, 512], F16, tag="cs", name="zero_sb")
        nc.vector.memset(zero_sb[:], 0.0)
        for p in range(2):
            for j in range(8):
                zero_writes[p].append(
                    nc.sync.dma_start(dsh[p].ap()[j], zero_sb[:]))

        # ================= layers (flat (l, b) unit pipeline) =================
        def load_qkvo(l):
            """Weight loads ride the GpSimd DMA queue (not Sync), so a load
            whose WAR-wait on the previous layer's last reads hasn't cleared
            can't head-of-line-block the Sync queue's shear DMAs. The qkvo
            load is emitted two units ahead (WAR on voT(l-1, last) clears
            early); the ffn load is emitted at unit (l, 0) itself since its
            WAR only clears at FFN2(l-1, last)."""
            qkvo_sb = wpool.tile([128, 4 * 2048], F16, tag="qkvo")
            nc.gpsimd.dma_start(
                qkvo_sb[:].rearrange("p (w k c) -> p w k c", w=4, k=4),
                wqkvo_e.ap()[l].rearrange("w k p c -> p w k c"))
            return qkvo_sb

        def load_ffn(l):
            ffn_sb = wpool.tile([128, 16384], F16, tag="ffn")
            nc.gpsimd.dma_start(
                ffn_sb[:, 0:8192].rearrange("p (k c) -> p k c", k=4),
                w1_e.ap()[l].rearrange("k p c -> p k c"))
            nc.gpsimd.dma_start(
                ffn_sb[:, 8192:16384].rearrange("p (k c) -> p k c", k=16),
                w2_e.ap()[l].rearrange("k p c -> p k c"))
            return ffn_sb

        if True:
            def stage_a(l, b, qkvo_sb):
                """PE-heavy front half: QKV gemms, Gram + shear write, voT.
                Emitted one batch AHEAD of stage_b(b-1) so the PE stream has
                queued work while b-1's correlation chain (DMA/DVE/ACT small
                ops) resolves."""
                h16 = cast_h16(b, "h16q")
                q16 = pool.tile([128, 4 * L], F16, tag="q16")
                k16 = pool.tile([128, 4 * L], F16, tag="k16")
                v16 = pool.tile([128, 4 * L], F16, tag="v16", name="v16")
                gemm_512(q16, 0, qkvo_sb, 0, h16, pq, bias_aps(l, 0))
                gemm_512(k16, 0, qkvo_sb, 2048, h16, pq, bias_aps(l, 1))
                gemm_512(v16, 0, qkvo_sb, 4096, h16, pq, bias_aps(l, 2))

                # ---- Gram M'_i = K_i^T Q (fp32 psum chunks, rounded to f16
                # per chunk); C'_j then summed from the f16 chunks on DVE:
                # C'_j[p,u] = sum_i M'_i[p, 128*((i+j)%8)+u]. A j-major
                # all-PSUM variant (32 128-wide matmuls per j) measured
                # +0.36ms: 128-wide moving exposes the 87ns LDWEIGHTS that a
                # 512-wide stream hides, so keep the 512-wide chunks.
                cacc = pool.tile([128, 1024], F32, tag="cacc")
                cw16 = pool.tile([128, 1024], F16, tag="cw16", name="cw16")
                for i in range(8):
                    for t2 in range(2):
                        psm = pq.tile([128, 512], F32, tag="ps")
                        for kc in range(4):
                            nc.tensor.matmul(
                                psm[:],
                                k16[:, L * kc + 128 * i: L * kc + 128 * i + 128],
                                q16[:, L * kc + 512 * t2: L * kc + 512 * t2 + 512],
                                start=(kc == 0), stop=(kc == 3))
                        chunk16 = pool.tile([128, 512], F16, tag="chunk16",
                                            bufs=2, name="chunk16")
                        nc.scalar.activation(chunk16[:], psm[:], AF.Identity)
                        for qb in range(4):
                            jq = 4 * t2 + qb          # q-time block in this chunk
                            j = (jq - i) % 8          # C'_j slice it feeds
                            blk = chunk16[:, 128 * qb:128 * qb + 128]
                            dst32 = cacc[:, 128 * j:128 * j + 128]
                            if i == 0:
                                nc.vector.tensor_copy(dst32, blk)
                            elif i < 7:
                                nc.vector.tensor_tensor(dst32, dst32, blk,
                                                        ALU.add)
                            else:
                                nc.vector.tensor_tensor(
                                    cw16[:, 128 * j:128 * j + 128], dst32, blk,
                                    ALU.add)
                # all 8 C'_j blocks in ONE 3-dim DMA (dst order (p, j, c))
                shw = bass.AP(tensor=dsh[b % 2].ap().tensor, offset=256,
                              ap=[[512, 128], [128 * 512, 8], [1, 128]])
                wr = nc.sync.dma_start(
                    shw, cw16[:].rearrange("p (j c) -> p j c", j=8))
                for j in range(8):
                    for prd in shear_reads[b % 2].get(j, []):
                        _add_dep_helper(wr.ins, prd.ins, sync=True,
                                        reason="shear WAR")
                    shear_writes[b % 2][j] = wr

                # voT = (v @ wo) TIME-major: voT_T[u, c] = sum_cin
                # v16[cin, 128T+u] * wo[cin, c]; stationary = v16 time-slices,
                # moving = wo [cin, 512 cout] (contiguous in qkvo_sb). bo is
                # dropped (decomp kills time-constant shifts, softmax sums
                # to 1). Emitted after the Gram so corr progress is preferred.
                vot = pool.tile([128, 8 * 512], F16, tag="vo2", bufs=2,
                                name="vot")
                for tb in range(8):
                    psv = pq.tile([128, 512], F32, tag="ps")
                    for kc in range(4):
                        nc.tensor.matmul(
                            psv[:],
                            v16[:, L * kc + 128 * tb: L * kc + 128 * tb + 128],
                            qkvo_sb[:, 6144 + 512 * kc: 6144 + 512 * kc + 512],
                            start=(kc == 0), stop=(kc == 3))
                    nc.scalar.activation(vot[:, 512 * tb:512 * tb + 512],
                                         psv[:], AF.Identity)
                return vot

            def stage_b_chain(b):
                # ---- shear read T[p, (j,u')] = D[j, p, p + 128 + u'] ----
                # ascending inner stride: 256 contiguous f16 = 512B/partition
                t16 = pool.tile([128, 8 * 256], F16, tag="t16", name="t16")
                shear_in = bass.AP(
                    tensor=dsh[b % 2].ap().tensor,
                    offset=128,
                    ap=[[513, 128], [128 * 512, 8], [1, 256]])
                rd = nc.sync.dma_start(
                    t16[:].rearrange("p (j t) -> p j t", j=8), shear_in)
                for j in range(8):
                    _add_dep_helper(rd.ins, shear_writes[b % 2][j].ins,
                                    sync=True, reason="shear RAW")
                    shear_reads[b % 2].setdefault(j, []).append(rd)
                for zw in zero_writes[b % 2]:
                    _add_dep_helper(rd.ins, zw.ins, sync=True,
                                    reason="shear zero RAW")

                # ---- s_j = ones(1/512)^T @ T_j  -> SBUF row (f32: the top-k
                # selection is precision-sensitive near ties) ----
                s_sb = pool.tile([1, 8 * 256], F32, tag="s_sb")
                for j2 in range(4):   # two j-blocks per 512-wide matmul
                    pss = pq.tile([1, 512], F32, tag="ps")
                    nc.tensor.matmul(pss[:], ones_sb[:],
                                     t16[:, 512 * j2:512 * j2 + 512],
                                     start=True, stop=True)
                    nc.scalar.activation(s_sb[:, 512 * j2:512 * j2 + 512],
                                         pss[:], AF.Identity)

                # ---- assemble r[128j+d] = sv_j[128+d] + sv_{j+1}[d] ----
                # (r32 shares the bmat slot: it is dead at max_index, before
                # this unit's bmat build; bmat(b) is dead once combine(b)'s
                # PE reads drain, well before r32(b+1) is written)
                r32 = pool.tile([1, L], F32, tag="bmat", name="r32")
                src_a = bass.AP(tensor=s_sb[:].tensor, offset=s_sb[:].offset + 128,
                                ap=[[s_sb[:].ap[0][0], 1], [256, 8], [1, 128]])
                nc.vector.tensor_copy(
                    r32[:].rearrange("p (j t) -> p j t", j=8), src_a)
                # += sv_{j+1}[d] for d>=1 ; j=0..6
                dst_b = bass.AP(tensor=r32[:].tensor, offset=r32[:].offset + 1,
                                ap=[[r32[:].ap[0][0], 1], [128, 7], [1, 127]])
                src_b = bass.AP(tensor=s_sb[:].tensor, offset=s_sb[:].offset + 257,
                                ap=[[s_sb[:].ap[0][0], 1], [256, 7], [1, 127]])
                nc.vector.tensor_tensor(dst_b, dst_b, src_b, ALU.add)
                # j=7 wraps to sv_0
                nc.vector.tensor_tensor(r32[:, 897:1024], r32[:, 897:1024],
                                        s_sb[:, 1:128], ALU.add)

                # ---- top-8 + softmax over top-6 ----
                top8 = pool.tile([1, 8], F32, tag="top8")
                idx8 = pool.tile([1, 8], DT.uint32, tag="idx8")
                nc.vector.max(top8[:], r32[:])
                nc.vector.max_index(idx8[:], top8[:], r32[:])
                negmax = pool.tile([1, 1], F32, tag="negmax")
                nc.vector.tensor_scalar_mul(negmax[:], top8[:, 0:1], -1.0)
                e6 = pool.tile([1, 8], F32, tag="e6")
                nc.scalar.activation(e6[:, 0:TOPK], top8[:, 0:TOPK], AF.Exp,
                                     bias=negmax[:], scale=1.0)
                se = pool.tile([1, 1], F32, tag="se")
                nc.vector.tensor_reduce(se[:], e6[:, 0:TOPK], AX.X, ALU.add)
                rse = pool.tile([1, 1], F32, tag="rse")
                nc.vector.reciprocal(rse[:], se[:])
                tc6 = pool.tile([1, 8], F32, tag="tc6")
                nc.vector.tensor_scalar_mul(tc6[:, 0:TOPK], e6[:, 0:TOPK], rse[:])
                # ---- broadcast tc + delays to all partitions via a rank-1
                # ones-row matmul (psum[p, j] = 1 * bcrow[0, j]) ----
                bcrow = pool.tile([1, 16], F32, tag="tcb", bufs=2)
                nc.vector.tensor_copy(bcrow[:, 0:TOPK], tc6[:, 0:TOPK])
                nc.vector.tensor_copy(bcrow[:, 8:8 + TOPK],
                                      idx8[0:1, 0:TOPK].bitcast(DT.int32))
                return bcrow

            def stage_b_chain2(b, bcrow):
                psbc = pq.tile([128, 16], F32, tag="ps")
                nc.tensor.matmul(psbc[:], onesr_sb[:], bcrow[:],
                                 start=True, stop=True)
                bc32 = pool.tile([128, 16], F32, tag="db8", bufs=2)
                nc.scalar.activation(bc32[:], psbc[:], AF.Identity)

                # ---- block-circulant shift matrix, 8 blocks of 128x128:
                # B[u, 128g+v] = sum_k tc_k * [(128g+u-v) mod 1024 == d_k]
                # (UG holds the mod-1024 iota; integers exact in f16) ----
                bmat = pool.tile([128, 1024], F16, tag="bmat", name="bmat")
                bsc = pool.tile([128, 1024], F16, tag="bsc", name="bsc")
                for k in range(TOPK):
                    dst = bmat if k == 0 else bsc
                    nc.vector.tensor_scalar(dst[:], ug_sb[:],
                                            bc32[:, 8 + k:9 + k],
                                            bc32[:, k:k + 1],
                                            ALU.is_equal, ALU.mult)
                    if k:
                        nc.vector.tensor_tensor(bmat[:], bmat[:], bsc[:],
                                                ALU.add)
                return bmat

            def stage_b_combine(b, vot, bmat):
                # ---- combine: h[c, 128T+v] += sum_g voT[(T+g)%8][u, c]^T
                # @ B_g[u, v], accumulated over g in PSUM ----
                for m in range(4):
                    for tb4 in range(2):
                        # four T-blocks share one psum bank (four accumulation
                        # groups in disjoint column quarters) so the h-update
                        # is one 512-wide DVE op instead of four 128-wide
                        psc = pq.tile([128, 512], F32, tag="ps")
                        for qtr in range(4):
                            tb = 4 * tb4 + qtr
                            for g in range(8):
                                j = (tb + g) % 8
                                nc.tensor.matmul(
                                    psc[:, 128 * qtr:128 * qtr + 128],
                                    vot[:, 512 * j + 128 * m:
                                        512 * j + 128 * m + 128],
                                    bmat[:, 128 * g:128 * g + 128],
                                    start=(g == 0), stop=(g == 7))
                        col = L * m + 512 * tb4
                        nc.vector.tensor_tensor(h_b[b][:, col:col + 512],
                                                h_b[b][:, col:col + 512],
                                                psc[:], ALU.add)

            def stage_b_back(l, b, ffn_sb):
                decomp(b)

                # ================= FFN =================
                # h16f reuses the h16q slot: h16q(b+1), cast in the pipelined
                # stage_a(b+1) emitted just before this, is dead once b+1's
                # QKV matmuls (earlier in the PE stream) have read it
                h16f = cast_h16(b, "h16q")
                for t in range(NTC):
                    # FFN intermediate lives in the dead q16/k16 slots (their
                    # tiles are consumed by stage_a's Gram, which for batch
                    # b+1 runs earlier in the PE stream than this FFN)
                    g16a = pool.tile([128, 8 * 512], F16, tag="q16",
                                     name="g16a")
                    g16b = pool.tile([128, 8 * 512], F16, tag="k16",
                                     name="g16b")
                    ghalf = [g16a, g16b]
                    for dm in range(16):
                        ps = pf.tile([128, 512], F32, tag="psf")
                        for k in range(4):
                            nc.tensor.matmul(
                                ps[:],
                                ffn_sb[:, 2048 * k + 128 * dm:
                                       2048 * k + 128 * dm + 128],
                                h16f[:, L * k + 512 * t: L * k + 512 * t + 512],
                                start=(k == 0), stop=(k == 3))
                        nc.scalar.activation(
                            ghalf[dm // 8][:, 512 * (dm % 8):
                                           512 * (dm % 8) + 512],
                            ps[:], AF.Gelu)
                    for m in range(4):
                        psy = pf.tile([128, 512], F32, tag="psf")
                        for k in range(16):
                            nc.tensor.matmul(
                                psy[:],
                                ffn_sb[:, 8192 + 512 * k + 128 * m:
                                       8192 + 512 * k + 128 * m + 128],
                                ghalf[k // 8][:, 512 * (k % 8):
                                              512 * (k % 8) + 512],
                                start=(k == 0), stop=(k == 15))
                        col = L * m + 512 * t
                        nc.vector.tensor_tensor(h_b[b][:, col:col + 512],
                                                h_b[b][:, col:col + 512],
                                                psy[:], ALU.add)
                decomp(b)

            def stage_final_stats(b):
                # ======== final my_Layernorm + projection for batch b ======
                hf = cast_h16(b, "h16q")
                # mu, sumsq rows via ones-matmuls (scaled by 1/512). stat
                # shares the cacc slot and mu2/sd the r32 slot: both tags'
                # pipelined uses (A-unit Gram / front r-assembly) interleave
                # in emission order with no cross-engine stalls.
                stat = pool.tile([1, 2 * L], F32, tag="cacc", name="stat")
                h2 = pool.tile([128, 4 * L], F16, tag="q16", name="h2")
                for m in range(4):
                    nc.scalar.activation(h2[:, L * m:L * m + L],
                                         hf[:, L * m:L * m + L], AF.Square)
                for which, src in ((0, hf), (1, h2)):
                    for t in range(NTC):
                        ps = pq.tile([1, 512], F32, tag="ps")
                        for k in range(4):
                            nc.tensor.matmul(
                                ps[:], ones_sb[:],
                                src[:, L * k + 512 * t: L * k + 512 * t + 512],
                                start=(k == 0), stop=(k == 3))
                        nc.vector.tensor_copy(
                            stat[:, which * L + 512 * t:
                                 which * L + 512 * t + 512], ps[:])
                # The projection is factored so its GEMMs depend only on hf,
                # never on a z-normalized tensor:
                #   out = P - mean_t(P) + pb,
                #   P[t,co] = inv[t]*A[t,co] - (inv*mu)[t]*colsum(pw)[co],
                #   A = hf^T @ pw.
                # The raw mu/ssq rows go time-major via a DRAM round trip;
                # the var/sqrt/recip/w math happens AFTER the transpose on
                # [128, 8] tiles (128 lanes) instead of 1-lane [1, 1024] rows.
                wr = nc.sync.dma_start(
                    dfin.ap()[b % 2].rearrange("r t -> (r t)"), stat[:])
                for prd in fin_reads.get(b % 2, []):
                    _add_dep_helper(wr.ins, prd.ins, sync=True,
                                    reason="fin WAR")
                # A[t, co] blocks: stationary = hf time-slices (ready at the
                # cast -- no wait on any normalization)
                a32 = pool.tile([128, 8 * 64], F32, tag="g16", name="a32")
                for tt in range(8):
                    ps = pf.tile([128, 512], F32, tag="psf")
                    for k in range(4):
                        nc.tensor.matmul(
                            ps[:, 0:64],
                            hf[:, L * k + 128 * tt: L * k + 128 * tt + 128],
                            projw_sb[:, 64 * k:64 * k + 64],
                            start=(k == 0), stop=(k == 3))
                    nc.scalar.activation(a32[:, 64 * tt:64 * tt + 64],
                                         ps[:, 0:64], AF.Identity)
                return a32, wr

            def stage_final_proj(b, a32, wr):
                # P assembly + time-mean subtraction + output store; emitted
                # one unit later so the DRAM round trip has fully landed
                invw = pool.tile([128, 16], F32, tag="db8", bufs=2,
                                 name="invw")
                for r in range(2):   # cols 0:8 = mu_t, 8:16 = ssq_t
                    rdap = bass.AP(tensor=dfin.ap().tensor,
                                   offset=2048 * (b % 2) + 1024 * r,
                                   ap=[[1, 128], [128, 8]])
                    rd = nc.sync.dma_start(invw[:, 8 * r:8 * r + 8], rdap)
                    _add_dep_helper(rd.ins, wr.ins, sync=True,
                                    reason="fin RAW")
                    fin_reads.setdefault(b % 2, []).append(rd)
                # inv = 1/sqrt(ssq - mu^2 + eps); w = inv*mu  (all [128, 8])
                iwt = pool.tile([128, 16], F32, tag="mz", name="iwt")
                nc.vector.tensor_tensor(iwt[:, 0:8], invw[:, 0:8],
                                        invw[:, 0:8], ALU.mult)
                nc.vector.tensor_tensor(iwt[:, 0:8], invw[:, 8:16],
                                        iwt[:, 0:8], ALU.subtract)
                nc.vector.tensor_scalar_add(iwt[:, 0:8], iwt[:, 0:8], EPS)
                nc.scalar.activation(iwt[:, 0:8], iwt[:, 0:8], AF.Sqrt)
                nc.vector.reciprocal(iwt[:, 0:8], iwt[:, 0:8])
                nc.vector.tensor_tensor(iwt[:, 8:16], iwt[:, 0:8],
                                        invw[:, 0:8], ALU.mult)
                for tt in range(8):
                    blk = a32[:, 64 * tt:64 * tt + 64]
                    nc.vector.tensor_scalar(blk, blk, iwt[:, tt:tt + 1],
                                            None, ALU.mult)
                    nc.vector.scalar_tensor_tensor(
                        blk, spwn_sb[:], iwt[:, 8 + tt:9 + tt], blk,
                        ALU.mult, ALU.add)
                # mean_t(P) row via ones(1/1024) matmuls, then pb - mean
                psm = pq.tile([1, 64], F32, tag="ps", padded_shape=[1, 512])
                for tt in range(8):
                    nc.tensor.matmul(psm[:], onesk_sb[:],
                                     a32[:, 64 * tt:64 * tt + 64],
                                     start=(tt == 0), stop=(tt == 7))
                pbm = pool.tile([1, 64], F32, tag="tcb", bufs=2, name="pbm")
                nc.vector.tensor_tensor(pbm[:], pbt_sb[0:1, :], psm[:],
                                        ALU.subtract)
                psb = pq.tile([128, 64], F32, tag="ps")
                nc.tensor.matmul(psb[:], onesr_sb[:], pbm[:],
                                 start=True, stop=True)
                o16 = pool.tile([128, 8 * 64], F16, tag="o32", name="o16")
                for tt in range(8):
                    nc.vector.tensor_tensor(o16[:, 64 * tt:64 * tt + 64],
                                            a32[:, 64 * tt:64 * tt + 64],
                                            psb[:], ALU.add)
                # one 3-dim DMA: out[b, 128*tt + p, co] <- o16[p, 64*tt + co]
                odst = bass.AP(tensor=out_e.ap().tensor, offset=b * L * 64,
                               ap=[[64, 128], [8192, 8], [1, 64]])
                nc.sync.dma_start(odst,
                                  o16[:].rearrange("p (t c) -> p t c", t=8))

            # -- software pipeline over ALL (l, b) units, 2 units deep: the
            # PE stream per unit is [combine(u)][QKV/Gram/voT(u+2)][FFN(u)],
            # so u's correlation chain (DMA/DVE/ACT) hides under u+1/u+2 PE
            # work, u's decomp latency hides under u+2's QKV/Gram, and layer
            # boundaries don't drain the pipeline (weights for l+1 load via
            # the gpsimd DMA queue two units ahead). The final my_Layernorm +
            # projection for batch b rides the tail of the last layer's unit.
            units = [(l, b) for l in range(nlayers) for b in range(nbatch)]
            vots, wmap = {}, {}

            def do_a(u):
                l, b = units[u]
                if b == 0:
                    wmap[l] = [load_qkvo(l), None]
                vots[u] = stage_a(l, b, wmap[l][0])

            # chain(u+1) is emitted a full unit before combine(u+1) so its
            # small-op DVE tail (r-assembly, top-8, softmax, B-build) runs
            # ahead of unit u's decomp work in the DVE queue and the combine
            # matmuls never wait on it; the shear-read DMA also stays ahead
            # of unit u+2's shear-writes in the Sync queue.
            do_a(0)
            if len(units) > 1:
                do_a(1)
            # finals are double-deferred: stats(b) runs after combine(u+1)
            # (so its matmuls never wait on decomp2(u)-gated casts ahead of
            # the combine), proj(b) after combine(u+2)
            bmats = {0: stage_b_chain2(0, stage_b_chain(0))}
            pending_stats = None
            pending_proj = None
            for u, (l, b) in enumerate(units):
                if b == 0:
                    wmap[l][1] = load_ffn(l)
                stage_b_combine(b, vots.pop(u), bmats.pop(u))
                if pending_proj is not None:
                    stage_final_proj(*pending_proj)
                    pending_proj = None
                if pending_stats is not None:
                    pending_proj = ((pending_stats,)
                                    + stage_final_stats(pending_stats))
                    pending_stats = None
                bcrow_n = (stage_b_chain(units[u + 1][1])
                           if u + 1 < len(units) else None)
                if u + 2 < len(units):
                    do_a(u + 2)
                if u + 1 < len(units):
                    bmats[u + 1] = stage_b_chain2(units[u + 1][1], bcrow_n)
                stage_b_back(l, b, wmap[l][1])
                if l == nlayers - 1:
                    pending_stats = b
            stage_final_proj(*pending_proj)
            stage_final_proj(pending_stats, *stage_final_stats(pending_stats))

    nc.compile()
    return nc


def _get_program(nbatch=4, nlayers=NL_TOT):
    key = (nbatch, nlayers)
    if key not in _BUILD_CACHE:
        _BUILD_CACHE[key] = _build(nbatch, nlayers)
    return _BUILD_CACHE[key]


def _prep_shared(inputs, nlayers):
    """Host-side input marshalling shared by all cores (weight layout/cast)."""
    f16 = np.float16
    wqkvo = np.stack([np.stack([np.asarray(inputs[n][l]).reshape(4, 128, 512)
                                for n in ("wq", "wk", "wv", "wo")])
                      for l in range(nlayers)]).astype(f16)
    w1 = np.stack([np.asarray(inputs["w1"][l]).reshape(4, 128, DFF)
                   for l in range(nlayers)]).astype(f16)
    w2 = np.stack([np.asarray(inputs["w2"][l]).reshape(16, 128, 512)
                   for l in range(nlayers)]).astype(f16)
    ew = np.asarray(inputs["embed_w"]).astype(f16)
    biases = np.zeros((128, nlayers * 16), np.float32)
    for l in range(nlayers):
        for w, n in enumerate(("bq", "bk", "bv", "bo")):
            arr = np.asarray(inputs[n][l])
            for m in range(4):
                biases[:, l * 16 + w * 4 + m] = arr[m * 128:(m + 1) * 128]
    pw_full = (np.asarray(inputs["ln_g"])[:, None]
               * np.asarray(inputs["proj_w"])).astype(np.float32)
    projw = pw_full.reshape(4, 128, 64).astype(f16)
    pbt = np.tile(np.asarray(inputs["proj_b"])[None, :], (128, 1)).astype(np.float32)
    # negated column sums of the (f16-rounded) ln_g-scaled projection, for
    # the factored final-LN: P = inv*A - (inv*mu)*colsum(pw)
    spwn = np.tile(-pw_full.astype(f16).astype(np.float32).sum(0)[None, :],
                   (128, 1)).astype(np.float32)
    onesk = np.full((128, 1), 1.0 / 1024, np.float32)
    # padded-cumsum edge ramps: front cs[i] = (i-12)*x0, back = total+k*xlast
    rampf = np.tile(np.arange(-PAD, 1, dtype=np.float32)[None, :], (128, 1))
    rampb = np.tile(np.arange(1, PAD + 1, dtype=np.float32)[None, :], (128, 1))
    ones512 = np.full((128, 1), 1.0 / 512, f16)
    # mod-1024 iota for the circulant shift blocks:
    # UG[u, 128g+v] = (128g + u - v) mod 1024 (integers <= 1023, f16-exact)
    u = np.arange(128)[:, None]
    v = np.arange(128)[None, :]
    ug = np.concatenate([(128 * g + u - v) % 1024 for g in range(8)],
                        axis=1).astype(f16)
    onesr = np.ones((1, 128), np.float32)
    return dict(wqkvo=wqkvo, w1=w1, w2=w2, ew=ew, biases=biases, projw=projw,
                pbt=pbt, rampf=rampf, rampb=rampb, ones512=ones512,
                ug=ug, onesr=onesr, spwn=spwn, onesk=onesk)


def _prep_x(xb):
    """(nb, L, CIN) fp32 -> (nb, 64, L+2) fp16 feature-major, circular padded."""
    xt = np.transpose(np.asarray(xb), (0, 2, 1))  # (nb, C, L)
    xe = np.concatenate([xt[:, :, -1:], xt, xt[:, :, :1]], axis=2)
    return xe.astype(np.float16)


def kernel(**inputs):
    from concourse.bass_utils import run_bass_kernel_spmd
    x = np.asarray(inputs["x"])
    B = x.shape[0]
    nbatch = B // NCORES
    nc = _get_program(nbatch, NL_TOT)
    shared = _prep_shared(inputs, NL_TOT)
    in_maps = []
    for c in range(NCORES):
        m = dict(shared)
        m["x"] = _prep_x(x[c * nbatch:(c + 1) * nbatch])
        in_maps.append(m)
    res = run_bass_kernel_spmd(nc, in_maps, core_ids=list(range(NCORES)))
    out = np.concatenate([res.results[c]["out"] for c in range(NCORES)], axis=0)
    return out.astype(np.float32)

